# revision 31
# baseline (speedup 1.0000x reference)
"""Adaptive LM head (3-tier chunked softmax cross-entropy) on 8 TRN2 NeuronCores.

Strategy: data-parallel over B_T = 8192 rows (1024 rows/core; weights
replicated, pre-cast to fp8 on the host). The kernel is bound by draining
softmax logits out of PSUM (exp + row-sum of 51.5M elements/core), so that
work is split across both PSUM-capable engines:
  - ScalarE: true exp via one merged activation(Exp, accum_out=...) per
    PSUM round (up to 1536 wide, amortizing the ~400ns instruction+
    accumulator-read overhead).
  - VectorE: Schraudolph fast-exp on tier-pure rounds: one tensor_scalar
    computes int16(round(l*2^7/ln2 + C2_tier)) == the bit pattern of
    bf16(e^l) (C2 calibrated per tier to zero the Z bias), then a bf16
    2x-mode tensor_tensor adds the bits-view into a per-row-tile
    accumulator.
  - TensorE: fp8 DoubleRow matmuls (tiers 0/1) and fp8 matmuls (tier 2)
    into a manually rotated [128,4096] f32 PSUM mega-tile; rounds are
    bank-aligned [1536,1536,1024] so three rounds are in flight.
  - target logit = dot(feature_row, W[:, target]) in bf16: indirect-DMA
    gather of transposed-weight rows + scalar_tensor_tensor multiply-
    reduce, spread through the main stream.
  - per-core partial loss (sum_rows(log Z - target_logit)/8192) is the
    output; the host sums the 8 partials.
"""

import numpy as np
import ml_dtypes

from concourse import bacc, bass, mybir
from concourse.bass import IndirectOffsetOnAxis
from concourse.bass_utils import run_bass_kernel_spmd
from concourse.tile import TileContext

F32 = mybir.dt.float32
BF16 = mybir.dt.bfloat16
I32 = mybir.dt.int32
I16 = mybir.dt.int16
FP8 = mybir.dt.float8e4
DR = mybir.MatmulPerfMode.DoubleRow
ALU = mybir.AluOpType
ACTF = mybir.ActivationFunctionType

P = 128
D = 1024
N_CORES = 8
RPC = 1024          # rows per core
NRT = RPC // P      # row tiles per core = 8
ST = 2048           # vocab super-tile width (DMA granularity)
GW = 512            # psum group width (one f32 PSUM bank)
V0, V1, V2 = 8192, 16384, 25681
PD1, PD2 = 256, 128
B_T = 8192
ZC = 64             # zbigA columns per row-tile
VW = 1536           # vacc columns per row-tile (max V-round width)

# PSUM mega-tile round rotation: bank-aligned offsets/widths.
CYCLE = ((0, 1536), (1536, 1536), (3072, 512))

# Schraudolph bf16 fast-exp: bits16(e^x) ~= round(x*C1 + C2_t).
# C2 calibrated per tier against the tier's logit sigma so the mean
# multiplicative bias of sum(exp) is ~0 (sigma: t0~0.64, t1~0.21, t2~0.15).
EXP_C1 = float(2.0 ** 7 / np.log(2.0))
EXP_C2 = (16256.0 - 7.25, 16256.0 - 6.5, 16256.0 - 5.5)

# DMA windows: (tier0 st, [tier1 sts], [tier2 sts]) of 2048-col supertiles.
WINDOWS = [
    (0, [0, 1], [0, 1, 2]),
    (1, [2, 3], [3, 4, 5]),
    (2, [4, 5], [6, 7, 8]),
    (3, [6, 7], [9, 10, 11, 12]),
]

G_RTS = ()          # row-tiles whose fast-exp accumulation runs on GpSimd
T_STS = (2, 4, 5, 7, 8)   # tier2 supertiles computed transposed (TE Z-sum)
ZBASE = 3584              # mega col of the reserved Z-accumulator bank

_NC_CACHE = None


def _ceil_div(a, b):
    return (a + b - 1) // b


def _cost_act(w):
    # merged ACTIVATE + ACTIVATION_READ_ACCUMULATOR + dispatch
    return (w + 222) / 1.2 + 181.0 + 30.0


def _cost_dve(w):
    # pass1 (1x from PSUM) + bf16 TT-add at 2x + dispatch
    return (w + 120) / 0.96 + (w / 2 + 58) / 0.96 + 120.0


def _build_graph():
    nc = bacc.Bacc("TRN2", target_bir_lowering=False, debug=False,
                   num_devices=N_CORES)

    ht_ext = nc.declare_dram_parameter("ht", [P, 8 * RPC], FP8, isOutput=False)
    hr_ext = nc.declare_dram_parameter("hr", [RPC, D], BF16, isOutput=False)
    tf_ext = nc.declare_dram_parameter("tf", [P, NRT], F32, isOutput=False)
    wp1_ext = nc.declare_dram_parameter("wp1", [P, 8 * PD1], FP8, isOutput=False)
    wp2_ext = nc.declare_dram_parameter("wp2", [P, 8 * PD2], FP8, isOutput=False)
    w0_ext = nc.declare_dram_parameter("w0", [D, V0], FP8, isOutput=False)
    w1_ext = nc.declare_dram_parameter("w1", [PD1, V1], FP8, isOutput=False)
    w2_ext = nc.declare_dram_parameter("w2", [PD2, V2], FP8, isOutput=False)
    wt0_ext = nc.declare_dram_parameter("wt0", [V0, D], BF16, isOutput=False)
    wt1_ext = nc.declare_dram_parameter("wt1", [V1, PD1], BF16, isOutput=False)
    wt2_ext = nc.declare_dram_parameter("wt2", [V2, PD2], BF16, isOutput=False)
    out_ext = nc.declare_dram_parameter("out", [1, 1], F32, isOutput=True)

    with TileContext(nc) as tc:
        with (
            tc.tile_pool(name="res", bufs=1) as res,
            tc.tile_pool(name="w0pool", bufs=2) as w0pool,
            tc.tile_pool(name="w1pool", bufs=4) as w1pool,
            tc.tile_pool(name="w2pool", bufs=6) as w2pool,
            tc.tile_pool(name="hrpool", bufs=2) as hrpool,
            tc.tile_pool(name="expool", bufs=3) as expool,
            tc.tile_pool(name="e16pool", bufs=4) as e16pool,
            tc.tile_pool(name="te16pool", bufs=4) as te16pool,
            tc.tile_pool(name="gpool", bufs=2) as gpool,
            tc.tile_pool(name="prodpool", bufs=2) as prodpool,
            tc.tile_pool(name="psum", bufs=1, space="PSUM") as psum,
        ):
            # ---------------- resident tiles ----------------
            ht8_sb = res.tile([P, 8 * RPC], FP8, tag="ht8")
            wp1_8 = res.tile([P, 8 * PD1], FP8, tag="wp18")
            wp2_8 = res.tile([P, 8 * PD2], FP8, tag="wp28")
            hp1T_sb = res.tile([P, 2 * RPC], FP8, tag="hp1T")
            hp2T_sb = res.tile([P, 1 * RPC], FP8, tag="hp2T")
            hp1r_sb = res.tile([P, NRT * PD1], BF16, tag="hp1r")
            hp2r_sb = res.tile([P, NRT * PD2], BF16, tag="hp2r")
            tf_sb = res.tile([P, NRT], F32, tag="tf")
            ge1 = res.tile([P, NRT], F32, tag="ge1")
            ge2 = res.tile([P, NRT], F32, tag="ge2")
            idxf = [res.tile([P, NRT], F32, tag=f"idxf{t}", name=f"idxf{t}")
                    for t in range(3)]
            idxi = [res.tile([P, NRT], I32, tag=f"idxi{t}", name=f"idxi{t}")
                    for t in range(3)]
            tl = [res.tile([P, NRT], F32, tag=f"tl{t}", name=f"tl{t}")
                  for t in range(3)]
            zbigA = res.tile([P, NRT * ZC], F32, tag="zbigA")
            vacc = res.tile([P, NRT * VW], BF16, tag="vacc")
            zredA = res.tile([P, NRT], F32, tag="zredA")
            zredV = res.tile([P, NRT], F32, tag="zredV")
            zred = res.tile([P, NRT], F32, tag="zred")
            logz = res.tile([P, NRT], F32, tag="logz")
            d1 = res.tile([P, NRT], F32, tag="d1")
            d2 = res.tile([P, NRT], F32, tag="d2")
            loss8 = res.tile([P, NRT], F32, tag="loss8")
            lossv = res.tile([P, 1], F32, tag="lossv")
            ones = res.tile([P, 1], F32, tag="ones")
            onesb = res.tile([P, 1], BF16, tag="onesb")
            zt_sb = res.tile([P, 8], F32, tag="zt_sb")
            ztp = res.tile([P, NRT], F32, tag="ztp")
            part = res.tile([1, 1], F32, tag="part")

            mega = psum.tile([P, 4096], F32, tag="mega")

            # fp8 staging: host pre-chunked layouts, split across the
            # HWDGE (sync) and SWDGE (gpsimd) queues so ht8 (1MB, gates
            # the hp2T projection) lands as early as possible
            nc.sync.dma_start(out=ht8_sb[:, 0:4 * RPC],
                              in_=ht_ext[:, 0:4 * RPC])
            nc.gpsimd.dma_start(out=ht8_sb[:, 4 * RPC:8 * RPC],
                                in_=ht_ext[:, 4 * RPC:8 * RPC])
            nc.sync.dma_start(out=wp2_8[:], in_=wp2_ext[:, :])
            nc.sync.dma_start(out=wp1_8[:], in_=wp1_ext[:, :])
            nc.sync.dma_start(out=tf_sb[:], in_=tf_ext[:, :])

            nc.vector.memset(zbigA[:], 0.0)
            nc.vector.memset(ones[:], 1.0)
            nc.vector.memset(onesb[:], 1.0)
            nc.vector.memset(ztp[:], 0.0)
            warm = res.tile([1, 1], F32, tag="warm")
            nc.scalar.activation(warm[0:1, 0:1], ones[0:1, 0:1], ACTF.Exp)
            # PE warmup: ~40 tiny matmuls on a garbage tile unthrottle the
            # HAM clock gate (~3.4us of activity) before the real stream
            junk = res.tile([P, 4 * P], FP8, tag="junk")
            nc.vector.memset(junk[:], 0.0)
            for wi_ in range(40):
                nc.tensor.matmul(
                    out=mega[:, 3072 + (wi_ % 2) * P:
                             3072 + (wi_ % 2) * P + P],
                    lhsT=junk[:, 0:P], rhs=junk[:, (wi_ % 3) * P:
                                                (wi_ % 3) * P + P],
                    start=True, stop=True)

            # ---------------- masks and in-tier indices ----------------
            nc.vector.tensor_scalar(out=ge1[:], in0=tf_sb[:], scalar1=float(V0),
                                    scalar2=None, op0=ALU.is_ge)
            nc.vector.tensor_scalar(out=ge2[:], in0=tf_sb[:],
                                    scalar1=float(V0 + V1), scalar2=None,
                                    op0=ALU.is_ge)
            nc.vector.tensor_scalar(out=idxf[0][:], in0=tf_sb[:],
                                    scalar1=float(V0 - 1), scalar2=None,
                                    op0=ALU.min)
            nc.vector.tensor_scalar(out=idxf[1][:], in0=tf_sb[:],
                                    scalar1=-float(V0), scalar2=0.0,
                                    op0=ALU.add, op1=ALU.max)
            nc.vector.tensor_scalar(out=idxf[1][:], in0=idxf[1][:],
                                    scalar1=float(V1 - 1), scalar2=None,
                                    op0=ALU.min)
            nc.vector.tensor_scalar(out=idxf[2][:], in0=tf_sb[:],
                                    scalar1=-float(V0 + V1), scalar2=0.0,
                                    op0=ALU.add, op1=ALU.max)
            nc.vector.tensor_scalar(out=idxf[2][:], in0=idxf[2][:],
                                    scalar1=float(V2 - 1), scalar2=None,
                                    op0=ALU.min)
            for t in range(3):
                nc.vector.tensor_copy(out=idxi[t][:], in_=idxf[t][:])

            ht8v = ht8_sb[:].rearrange("p (k r) -> p k r", k=8)
            wp18v = wp1_8[:].rearrange("p (k c) -> p k c", k=8)
            wp28v = wp2_8[:].rearrange("p (k c) -> p k c", k=8)
            hp1Tv = hp1T_sb[:].rearrange("p (k r) -> p k r", k=2)

            # round-slot rotation over the mega tile
            slot_i = [0]

            def next_slot():
                off, w = CYCLE[slot_i[0] % 3]
                slot_i[0] += 1
                return off, w

            def next_slot_min(minw):
                while True:
                    off, w = next_slot()
                    if w >= minw:
                        return off, w

            # greedy engine-balance state (ns)
            eng_t = {"A": 0.0, "V": 12000.0}
            zcols = [0] * NRT

            # ---------------- hp2T projection (runway prerequisite) -------
            base, cap = next_slot_min(RPC)
            for g in range(2):
                for pr in range(4):
                    nc.tensor.matmul(
                        out=mega[:, base + g * GW: base + (g + 1) * GW],
                        lhsT=wp28v[:, 2 * pr: 2 * pr + 2, 0:P],
                        rhs=ht8v[:, 2 * pr: 2 * pr + 2, g * GW:(g + 1) * GW],
                        start=(pr == 0), stop=(pr == 3), perf_mode=DR)
            nc.vector.tensor_copy(out=hp2T_sb[:],
                                  in_=mega[:, base: base + RPC])
            eng_t["V"] += (RPC + 120) / 0.96 + 60

            # ---------------- main stream ----------------
            # tier -> (V, Kchunks, w_ext, wpool, doublerow)
            tiers = {
                0: (V0, 8, w0_ext, w0pool, True),
                1: (V1, 2, w1_ext, w1pool, True),
                2: (V2, 1, w2_ext, w2pool, False),
            }
            gather_src = [wt0_ext, wt1_ext, wt2_ext]
            gdim = [D, PD1, PD2]
            gmax = [V0 - 1, V1 - 1, V2 - 1]
            st_wtile = {}

            def ensure_st(tier, st):
                if (tier, st) in st_wtile:
                    return
                V, K, w_ext, wpool, dr = tiers[tier]
                w = min(ST, V - st * ST)
                wtile = wpool.tile([P, K * ST], FP8,
                                   tag=f"w{tier}", name=f"w{tier}")
                for k in range(K):
                    nc.gpsimd.dma_start(
                        out=wtile[:, k * ST: k * ST + w],
                        in_=w_ext[k * P:(k + 1) * P, st * ST: st * ST + w])
                st_wtile[(tier, st)] = wtile

            def st_groups(tier, st):
                V = tiers[tier][0]
                w = min(ST, V - st * ST)
                return [(tier, st, g, min(GW, w - g * GW))
                        for g in range(_ceil_div(w, GW))]

            def emit_round(groups, rt, useV):
                base, cap = next_slot()
                flush_ones()
                off = 0
                for (tier, st, g, gw) in groups:
                    V, K, w_ext, wpool, dr = tiers[tier]
                    wtile = st_wtile[(tier, st)]
                    dst = mega[:, base + off: base + off + gw]
                    if dr:
                        wv = wtile[:].rearrange("p (k c) -> p k c", k=K)
                        lv = ht8v if tier == 0 else hp1Tv
                        for pr in range(K // 2):
                            nc.tensor.matmul(
                                out=dst,
                                lhsT=lv[:, 2 * pr: 2 * pr + 2,
                                        rt * P: rt * P + P],
                                rhs=wv[:, 2 * pr: 2 * pr + 2,
                                       g * GW: g * GW + gw],
                                start=(pr == 0), stop=(pr == K // 2 - 1),
                                perf_mode=DR)
                    else:
                        nc.tensor.matmul(
                            out=dst,
                            lhsT=hp2T_sb[:, rt * P: rt * P + P],
                            rhs=wtile[:, g * GW: g * GW + gw],
                            start=True, stop=True)
                    off += gw
                src = mega[:, base: base + off]
                if useV:
                    tier = groups[0][0]
                    e16 = e16pool.tile([P, 1536], I16, tag="e16")
                    nc.vector.tensor_scalar(
                        out=e16[:, :off], in0=src,
                        scalar1=EXP_C1, scalar2=EXP_C2[tier],
                        op0=ALU.mult, op1=ALU.add)
                    va = vacc[:, rt * VW: rt * VW + off]
                    if rt in G_RTS:
                        nc.gpsimd.tensor_tensor(
                            out=va, in0=va, in1=e16[:, :off].bitcast(BF16),
                            op=ALU.add)
                        eng_t["V"] += (off + 120) / 0.96 + 60.0
                    else:
                        nc.vector.tensor_tensor(
                            out=va, in0=va, in1=e16[:, :off].bitcast(BF16),
                            op=ALU.add)
                        eng_t["V"] += _cost_dve(off)
                else:
                    zcol = rt * ZC + zcols[rt]
                    zcols[rt] += 1
                    ex = expool.tile([P, 1536], BF16, tag="ex")
                    nc.scalar.activation(
                        ex[:, :off], src, ACTF.Exp,
                        accum_out=zbigA[:, zcol: zcol + 1])
                    eng_t["A"] += _cost_act(off)

            last_eng = ["A"]

            # transposed tier-2 units: (st, vslice, rowblock)
            trans_q = [(st, v, b) for st in T_STS for v in range(16)
                       for b in (0, 1)]
            n_trans = len(trans_q)
            trans_state = {"emitted": 0, "first": True}
            pending_ones = []

            def flush_ones(all_=False):
                while len(pending_ones) >= (1 if all_ else 2):
                    for (e16t, uoff, blk) in pending_ones.pop(0):
                        trans_state["emitted"] += 1
                        nc.tensor.matmul(
                            out=mega[blk * 32: blk * 32 + 1,
                                     ZBASE: ZBASE + GW],
                            lhsT=onesb[:, 0:1],
                            rhs=e16t[:, uoff: uoff + GW].bitcast(BF16),
                            start=trans_state["first"],
                            stop=(trans_state["emitted"] == n_trans),
                            skip_group_check=True)
                        trans_state["first"] = False

            def emit_trans_round():
                cap_next = CYCLE[slot_i[0] % 3][1]
                avail = trans_avail()
                n = min(cap_next // GW, len(avail))
                if n == 0:
                    return False
                units = avail[:n]
                for u in units:
                    trans_q.remove(u)
                base, cap = next_slot()
                flush_ones()
                for u, (st, v, b) in enumerate(units):
                    wtile = st_wtile[(2, st)]
                    nc.tensor.matmul(
                        out=mega[:, base + u * GW: base + (u + 1) * GW],
                        lhsT=wtile[:, v * P:(v + 1) * P],
                        rhs=hp2T_sb[:, b * GW:(b + 1) * GW],
                        start=True, stop=True)
                off = len(units) * GW
                e16 = te16pool.tile([P, 1536], I16, tag="te16")
                nc.vector.tensor_scalar(
                    out=e16[:, :off], in0=mega[:, base: base + off],
                    scalar1=EXP_C1, scalar2=EXP_C2[2],
                    op0=ALU.mult, op1=ALU.add)
                eng_t["V"] += (off + 120) / 0.96 + 60.0
                pending_ones.append(
                    [(e16, u * GW, b) for u, (st, v, b) in enumerate(units)])
                return True

            def trans_avail():
                return [u for u in trans_q if (2, u[0]) in st_wtile]

            def plan_emit(tier_lists, rt):
                # per-tier queues; rounds draw via largest-remaining-fraction
                qs = [list(l) for l in tier_lists if l]
                tot = [len(q) for q in qs]
                while any(qs):
                    cap = CYCLE[slot_i[0] % 3][1]
                    if cap == GW and emit_trans_round():
                        continue
                    nfit = cap // GW
                    # tier-pure candidate for a V round: tier with the most
                    # remaining groups
                    vi = max(range(len(qs)), key=lambda j: len(qs[j]))
                    vgroups = qs[vi][:nfit]
                    # A-round candidate: Bresenham across tiers
                    apick = []
                    idx = [0] * len(qs)
                    for _ in range(nfit):
                        best, bj = -1.0, -1
                        for j, q in enumerate(qs):
                            rema = len(q) - idx[j]
                            if rema > 0 and rema / tot[j] > best:
                                best, bj = rema / tot[j], j
                        if bj < 0:
                            break
                        apick.append((bj, idx[bj]))
                        idx[bj] += 1
                    agroups = [qs[j][k] for (j, k) in apick]
                    wV = sum(g[3] for g in vgroups)
                    wA = sum(g[3] for g in agroups)
                    # alternation bias against same-engine streaks
                    bias = 250.0 if last_eng[0] == "A" else -250.0
                    useV = bool(vgroups) and (
                        eng_t["V"] + _cost_dve(wV) - bias <
                        eng_t["A"] + _cost_act(wA))
                    if useV:
                        qs[vi] = qs[vi][nfit:]
                        emit_round(vgroups, rt, True)
                        last_eng[0] = "V"
                    else:
                        for (j, k) in sorted(apick, reverse=True):
                            qs[j].pop(k)
                        emit_round(agroups, rt, False)
                        last_eng[0] = "A"

            def emit_rt_final(rt):
                # row-tile Z reduction, emitted as soon as rt's stream ends
                nc.vector.tensor_reduce(
                    out=zredA[:, rt:rt + 1],
                    in_=zbigA[:, rt * ZC:(rt + 1) * ZC],
                    axis=mybir.AxisListType.X, op=ALU.add)
                eng_t["V"] += (ZC + 58) / 0.96 + 60
                cA = (VW + 224) / 1.2 + 181
                cV = (VW + 58) / 0.96
                if eng_t["A"] + cA < eng_t["V"] + cV:
                    ex = expool.tile([P, 1536], BF16, tag="ex")
                    nc.scalar.activation(
                        ex[:, :VW], vacc[:, rt * VW:(rt + 1) * VW],
                        ACTF.Identity, accum_out=zredV[:, rt:rt + 1])
                    eng_t["A"] += cA
                else:
                    nc.vector.tensor_reduce(
                        out=zredV[:, rt:rt + 1],
                        in_=vacc[:, rt * VW:(rt + 1) * VW],
                        axis=mybir.AxisListType.X, op=ALU.add)
                    eng_t["V"] += cV

            def emit_rows_proj(rt, t):
                # DR rows-orientation projection feeding the target dot
                pd = PD1 if t == 1 else PD2
                wv = wp18v if t == 1 else wp28v
                dstt = hp1r_sb if t == 1 else hp2r_sb
                base, cap = next_slot()
                for pr in range(4):
                    nc.tensor.matmul(
                        out=mega[:, base: base + pd],
                        lhsT=ht8v[:, 2 * pr: 2 * pr + 2,
                                  rt * P: rt * P + P],
                        rhs=wv[:, 2 * pr: 2 * pr + 2, 0:pd],
                        start=(pr == 0), stop=(pr == 3), perf_mode=DR)
                nc.vector.tensor_copy(
                    out=dstt[:, rt * pd:(rt + 1) * pd],
                    in_=mega[:, base: base + pd])
                eng_t["V"] += (pd + 120) / 0.96 + 60

            def emit_gather_dot(i):
                rt, t = divmod(i, 3)
                if t == 0:
                    hr_t = hrpool.tile([P, D], BF16, tag="hrt", name="hrt")
                    nc.sync.dma_start(out=hr_t[:],
                                      in_=hr_ext[rt * P:(rt + 1) * P, :])
                    feat_ap = hr_t[:]
                elif t == 1:
                    emit_rows_proj(rt, 1)
                    feat_ap = hp1r_sb[:, rt * PD1:(rt + 1) * PD1]
                else:
                    emit_rows_proj(rt, 2)
                    feat_ap = hp2r_sb[:, rt * PD2:(rt + 1) * PD2]
                g = gpool.tile([P, gdim[t]], BF16, tag=f"g{t}", name=f"g{t}")
                nc.gpsimd.indirect_dma_start(
                    out=g[:], out_offset=None,
                    in_=gather_src[t][:, :],
                    in_offset=IndirectOffsetOnAxis(
                        ap=idxi[t][:, rt:rt + 1], axis=0),
                    bounds_check=gmax[t], oob_is_err=False)
                prod = prodpool.tile([P, D], BF16, tag="prod")
                nc.vector.scalar_tensor_tensor(
                    out=prod[:, :gdim[t]],
                    in0=feat_ap, scalar=1.0, in1=g[:],
                    op0=ALU.mult, op1=ALU.mult,
                    accum_out=tl[t][:, rt:rt + 1])
                eng_t["V"] += (gdim[t] / 2 + 58) / 0.96 + 60

            def interleave(lists):
                # Bresenham-style proportional merge of per-tier group lists
                out = []
                idx = [0] * len(lists)
                tot = [len(l) for l in lists]
                n = sum(tot)
                for _ in range(n):
                    best, bi = -1.0, 0
                    for j, l in enumerate(lists):
                        if idx[j] < tot[j]:
                            frac = (tot[j] - idx[j]) / tot[j]
                            if frac > best:
                                best, bi = frac, j
                    out.append(lists[bi][idx[bi]])
                    idx[bi] += 1
                return out

            gi = 0
            for wi, (a_st, b_sts, c_sts) in enumerate(WINDOWS):
                for st in c_sts:
                    ensure_st(2, st)
                ensure_st(0, a_st)
                for st in b_sts:
                    ensure_st(1, st)
                As = st_groups(0, a_st)
                Bs = [g for st in b_sts for g in st_groups(1, st)]
                Cs = [g for st in c_sts if st not in T_STS
                      for g in st_groups(2, st)]
                if wi == 0:
                    # runway: tier2 rounds only while w0/w1 land; vacc
                    # slices are zeroed here (V is otherwise idle early)
                    for rt in range(NRT):
                        if rt in G_RTS:
                            nc.gpsimd.memset(
                                vacc[:, rt * VW:(rt + 1) * VW], 0.0)
                        else:
                            nc.vector.memset(
                                vacc[:, rt * VW:(rt + 1) * VW], 0.0)
                        plan_emit([Cs[0:8]], rt)
                    # hp1T projection: needed by the first B rounds
                    for m in range(2):
                        base, cap = next_slot_min(RPC)
                        for g in range(2):
                            for pr in range(4):
                                nc.tensor.matmul(
                                    out=mega[:, base + g * GW:
                                             base + (g + 1) * GW],
                                    lhsT=wp18v[:, 2 * pr: 2 * pr + 2,
                                               m * P:(m + 1) * P],
                                    rhs=ht8v[:, 2 * pr: 2 * pr + 2,
                                             g * GW:(g + 1) * GW],
                                    start=(pr == 0), stop=(pr == 3),
                                    perf_mode=DR)
                        nc.vector.tensor_copy(
                            out=hp1T_sb[:, m * RPC:(m + 1) * RPC],
                            in_=mega[:, base: base + RPC])
                        eng_t["V"] += (RPC + 120) / 0.96 + 60
                    for rt in range(NRT):
                        plan_emit([Cs[8:], As, Bs], rt)
                    continue
                for rt in range(NRT):
                    plan_emit([As, Bs, Cs], rt)
                    for _ in range(2):
                        if trans_avail() and eng_t["V"] < eng_t["A"]:
                            emit_trans_round()
                    if gi < 3 * NRT:
                        emit_gather_dot(gi)
                        gi += 1
                    if wi == 3:
                        emit_rt_final(rt)
            while trans_q:
                if not emit_trans_round():
                    raise RuntimeError(f"stuck: {len(trans_q)} units left")
            flush_ones(all_=True)
            while gi < 3 * NRT:
                emit_gather_dot(gi)
                gi += 1
            # transposed-Z finale: PSUM row-vectors -> DRAM -> per-partition
            ztmp = res.tile([P, GW], F32, tag="ztmp")
            zdram = nc.dram_tensor("zscratch", [2, GW], F32, kind="Internal")
            for b in (0, 1):
                nc.vector.tensor_copy(
                    out=ztmp[b * 32: b * 32 + 1, :],
                    in_=mega[b * 32: b * 32 + 1, ZBASE: ZBASE + GW])
                nc.sync.dma_start(
                    out=zdram[b: b + 1, :],
                    in_=ztmp[b * 32: b * 32 + 1, :])
            nc.sync.dma_start(
                out=ztp[:, :],
                in_=zdram[:, :].rearrange("b (r p) -> p (b r)", p=P))

            # ---------------- final reduction ----------------
            # zred = zredA + zredV + d1 (d1 holds the ScalarE-reduced
            # second vacc half where that path was taken)
            nc.vector.tensor_tensor(out=zred[:], in0=zredA[:], in1=zredV[:],
                                    op=ALU.add)
            nc.vector.tensor_tensor(out=zred[:], in0=zred[:], in1=ztp[:],
                                    op=ALU.add)
            nc.scalar.activation(logz[:], zred[:], ACTF.Ln)
            # loss8 = logz - (tl0 + ge1*(tl1-tl0) + ge2*(tl2-tl1))
            nc.vector.tensor_tensor(out=d1[:], in0=tl[1][:], in1=tl[0][:],
                                    op=ALU.subtract)
            nc.vector.tensor_tensor(out=d2[:], in0=tl[2][:], in1=tl[1][:],
                                    op=ALU.subtract)
            nc.vector.tensor_tensor(out=d1[:], in0=d1[:], in1=ge1[:],
                                    op=ALU.mult)
            nc.vector.tensor_tensor(out=d2[:], in0=d2[:], in1=ge2[:],
                                    op=ALU.mult)
            nc.vector.tensor_tensor(out=loss8[:], in0=logz[:], in1=tl[0][:],
                                    op=ALU.subtract)
            nc.vector.tensor_tensor(out=loss8[:], in0=loss8[:], in1=d1[:],
                                    op=ALU.subtract)
            nc.vector.tensor_tensor(out=loss8[:], in0=loss8[:], in1=d2[:],
                                    op=ALU.subtract)
            nc.vector.tensor_reduce(out=lossv[:], in_=loss8[:],
                                    axis=mybir.AxisListType.X, op=ALU.add)
            base, cap = next_slot()
            nc.tensor.matmul(out=mega[0:1, base:base + 1], lhsT=lossv[:],
                             rhs=ones[:], start=True, stop=True)
            nc.scalar.mul(part[0:1, 0:1], mega[0:1, base:base + 1],
                          1.0 / float(B_T))
            nc.sync.dma_start(out=out_ext[:, :], in_=part[:])

    nc.compile()
    return nc


def _get_nc():
    global _NC_CACHE
    if _NC_CACHE is None:
        _NC_CACHE = _build_graph()
    return _NC_CACHE


def _make_in_maps(h, targets, W_head0, W_proj1, W_head1, W_proj2, W_head2):
    FP8NP = ml_dtypes.float8_e4m3
    BF16NP = ml_dtypes.bfloat16
    h = np.ascontiguousarray(np.asarray(h, dtype=np.float32)).reshape(B_T, D)
    t = np.asarray(targets).reshape(-1).astype(np.float32)
    w0 = np.asarray(W_head0, dtype=np.float32)
    w1 = np.asarray(W_head1, dtype=np.float32)
    w2 = np.asarray(W_head2, dtype=np.float32)
    wp1 = np.asarray(W_proj1, dtype=np.float32)
    wp2 = np.asarray(W_proj2, dtype=np.float32)
    w0_8 = np.ascontiguousarray(w0.astype(FP8NP))
    w1_8 = np.ascontiguousarray(w1.astype(FP8NP))
    w2_8 = np.ascontiguousarray(w2.astype(FP8NP))
    wp1_c = np.ascontiguousarray(
        wp1.astype(FP8NP).reshape(8, P, PD1).transpose(1, 0, 2).reshape(
            P, 8 * PD1))
    wp2_c = np.ascontiguousarray(
        wp2.astype(FP8NP).reshape(8, P, PD2).transpose(1, 0, 2).reshape(
            P, 8 * PD2))
    wt0 = np.ascontiguousarray(w0.T.astype(BF16NP))
    wt1 = np.ascontiguousarray(w1.T.astype(BF16NP))
    wt2 = np.ascontiguousarray(w2.T.astype(BF16NP))

    in_maps = []
    for c in range(N_CORES):
        hc = h[c * RPC:(c + 1) * RPC]
        tc_ = t[c * RPC:(c + 1) * RPC]
        ht8 = hc.T.astype(FP8NP).reshape(8, P, RPC).transpose(1, 0, 2)
        in_maps.append({
            "ht": np.ascontiguousarray(ht8.reshape(P, 8 * RPC)),
            "hr": np.ascontiguousarray(hc.astype(BF16NP)),
            "tf": np.ascontiguousarray(tc_.reshape(NRT, P).T),
            "wp1": wp1_c, "wp2": wp2_c,
            "w0": w0_8, "w1": w1_8, "w2": w2_8,
            "wt0": wt0, "wt1": wt1, "wt2": wt2,
        })
    return in_maps


def _finalize(results):
    total = sum(float(results[c]["out"][0, 0]) for c in range(N_CORES))
    return np.float32(total)


def kernel(h, targets, token_to_tier, token_to_idx,
           W_head0, W_proj1, W_head1, W_proj2, W_head2):
    in_maps = _make_in_maps(h, targets, W_head0, W_proj1, W_head1,
                            W_proj2, W_head2)
    nc = _get_nc()
    res = run_bass_kernel_spmd(nc, in_maps, core_ids=list(range(N_CORES)))
    return _finalize(res.results)


# revision 32
# speedup vs baseline: 1.1007x; 1.1007x over previous
"""Adaptive LM head (3-tier chunked softmax cross-entropy) on 8 TRN2 NeuronCores.

Strategy: data-parallel over B_T = 8192 rows (1024 rows/core; weights
replicated, pre-cast to fp8 on the host). The kernel is bound by draining
softmax logits out of PSUM (exp + row-sum of 51.5M elements/core), so that
work is split across both PSUM-capable engines:
  - ScalarE: true exp via one merged activation(Exp, accum_out=...) per
    PSUM round (up to 1536 wide, amortizing the ~400ns instruction+
    accumulator-read overhead).
  - VectorE: Schraudolph fast-exp on tier-pure rounds: one tensor_scalar
    computes int16(round(l*2^7/ln2 + C2_tier)) == the bit pattern of
    bf16(e^l) (C2 calibrated per tier to zero the Z bias), then a bf16
    2x-mode tensor_tensor adds the bits-view into a per-row-tile
    accumulator.
  - TensorE: fp8 DoubleRow matmuls (tiers 0/1) and fp8 matmuls (tier 2)
    into a manually rotated [128,4096] f32 PSUM mega-tile; rounds are
    bank-aligned [1536,1536,1024] so three rounds are in flight.
  - target logit = dot(feature_row, W[:, target]) in bf16: indirect-DMA
    gather of transposed-weight rows + scalar_tensor_tensor multiply-
    reduce, spread through the main stream.
  - per-core partial loss (sum_rows(log Z - target_logit)/8192) is the
    output; the host sums the 8 partials.
"""

import numpy as np
import ml_dtypes

from concourse import bacc, bass, mybir
from concourse.bass import IndirectOffsetOnAxis
from concourse.bass_utils import run_bass_kernel_spmd
from concourse.tile import TileContext

F32 = mybir.dt.float32
BF16 = mybir.dt.bfloat16
I32 = mybir.dt.int32
I16 = mybir.dt.int16
FP8 = mybir.dt.float8e4
DR = mybir.MatmulPerfMode.DoubleRow
ALU = mybir.AluOpType
ACTF = mybir.ActivationFunctionType

P = 128
D = 1024
N_CORES = 8
RPC = 1024          # rows per core
NRT = RPC // P      # row tiles per core = 8
ST = 2048           # vocab super-tile width (DMA granularity)
GW = 512            # psum group width (one f32 PSUM bank)
V0, V1, V2 = 8192, 16384, 25681
PD1, PD2 = 256, 128
B_T = 8192
ZC = 64             # zbigA columns per row-tile
VW = 1536           # vacc columns per row-tile (max V-round width)

# PSUM mega-tile round rotation: bank-aligned offsets/widths.
CYCLE = ((0, 1536), (1536, 1536), (3072, 512))

# Schraudolph bf16 fast-exp: bits16(e^x) ~= round(x*C1 + C2_t).
# C2 calibrated per tier against the tier's logit sigma so the mean
# multiplicative bias of sum(exp) is ~0 (sigma: t0~0.64, t1~0.21, t2~0.15).
EXP_C1 = float(2.0 ** 7 / np.log(2.0))
EXP_C2 = (16256.0 - 7.25, 16256.0 - 6.5, 16256.0 - 5.5)

# DMA windows: (tier0 st, [tier1 sts], [tier2 sts]) of 2048-col supertiles.
WINDOWS = [
    (0, [0, 1], [0, 1, 2]),
    (1, [2, 3], [3, 4, 5]),
    (2, [4, 5], [6, 7, 8]),
    (3, [6, 7], [9, 10, 11, 12]),
]

G_RTS = ()          # row-tiles whose fast-exp accumulation runs on GpSimd
T_STS = (2, 4, 5)   # tier2 supertiles computed transposed (TE Z-sum)
ZBASE = 3584              # mega col of the reserved Z-accumulator bank

_NC_CACHE = None


def _ceil_div(a, b):
    return (a + b - 1) // b


def _cost_act(w):
    # merged ACTIVATE + ACTIVATION_READ_ACCUMULATOR + dispatch
    return (w + 222) / 1.2 + 181.0 + 30.0


def _cost_dve(w):
    # pass1 (1x from PSUM) + bf16 TT-add at 2x + dispatch
    return (w + 120) / 0.96 + (w / 2 + 58) / 0.96 + 120.0


def _build_graph():
    nc = bacc.Bacc("TRN2", target_bir_lowering=False, debug=False,
                   num_devices=N_CORES)

    ht_ext = nc.declare_dram_parameter("ht", [P, 8 * RPC], FP8, isOutput=False)
    hr_ext = nc.declare_dram_parameter("hr", [RPC, D], BF16, isOutput=False)
    tf_ext = nc.declare_dram_parameter("tf", [P, NRT], F32, isOutput=False)
    wp1_ext = nc.declare_dram_parameter("wp1", [P, 8 * PD1], FP8, isOutput=False)
    wp2_ext = nc.declare_dram_parameter("wp2", [P, 8 * PD2], FP8, isOutput=False)
    w0_ext = nc.declare_dram_parameter("w0", [D, V0], FP8, isOutput=False)
    w1_ext = nc.declare_dram_parameter("w1", [PD1, V1], FP8, isOutput=False)
    w2_ext = nc.declare_dram_parameter("w2", [PD2, V2], FP8, isOutput=False)
    wt0_ext = nc.declare_dram_parameter("wt0", [V0, D], BF16, isOutput=False)
    wt1_ext = nc.declare_dram_parameter("wt1", [V1, PD1], BF16, isOutput=False)
    wt2_ext = nc.declare_dram_parameter("wt2", [V2, PD2], BF16, isOutput=False)
    out_ext = nc.declare_dram_parameter("out", [1, 1], F32, isOutput=True)

    with TileContext(nc) as tc:
        with (
            tc.tile_pool(name="res", bufs=1) as res,
            tc.tile_pool(name="w0pool", bufs=2) as w0pool,
            tc.tile_pool(name="w1pool", bufs=4) as w1pool,
            tc.tile_pool(name="w2pool", bufs=6) as w2pool,
            tc.tile_pool(name="hrpool", bufs=2) as hrpool,
            tc.tile_pool(name="expool", bufs=3) as expool,
            tc.tile_pool(name="e16pool", bufs=4) as e16pool,
            tc.tile_pool(name="te16pool", bufs=6) as te16pool,
            tc.tile_pool(name="gpool", bufs=2) as gpool,
            tc.tile_pool(name="prodpool", bufs=2) as prodpool,
            tc.tile_pool(name="psum", bufs=1, space="PSUM") as psum,
        ):
            # ---------------- resident tiles ----------------
            ht8_sb = res.tile([P, 8 * RPC], FP8, tag="ht8")
            wp1_8 = res.tile([P, 8 * PD1], FP8, tag="wp18")
            wp2_8 = res.tile([P, 8 * PD2], FP8, tag="wp28")
            hp1T_sb = res.tile([P, 2 * RPC], FP8, tag="hp1T")
            hp2T_sb = res.tile([P, 1 * RPC], FP8, tag="hp2T")
            hp1r_sb = res.tile([P, NRT * PD1], BF16, tag="hp1r")
            hp2r_sb = res.tile([P, NRT * PD2], BF16, tag="hp2r")
            tf_sb = res.tile([P, NRT], F32, tag="tf")
            ge1 = res.tile([P, NRT], F32, tag="ge1")
            ge2 = res.tile([P, NRT], F32, tag="ge2")
            idxf = [res.tile([P, NRT], F32, tag=f"idxf{t}", name=f"idxf{t}")
                    for t in range(3)]
            idxi = [res.tile([P, NRT], I32, tag=f"idxi{t}", name=f"idxi{t}")
                    for t in range(3)]
            tl = [res.tile([P, NRT], F32, tag=f"tl{t}", name=f"tl{t}")
                  for t in range(3)]
            zbigA = res.tile([P, NRT * ZC], F32, tag="zbigA")
            vacc = res.tile([P, NRT * VW], BF16, tag="vacc")
            zredA = res.tile([P, NRT], F32, tag="zredA")
            zredV = res.tile([P, NRT], F32, tag="zredV")
            zred = res.tile([P, NRT], F32, tag="zred")
            logz = res.tile([P, NRT], F32, tag="logz")
            d1 = res.tile([P, NRT], F32, tag="d1")
            d2 = res.tile([P, NRT], F32, tag="d2")
            loss8 = res.tile([P, NRT], F32, tag="loss8")
            lossv = res.tile([P, 1], F32, tag="lossv")
            ones = res.tile([P, 1], F32, tag="ones")
            onesb = res.tile([P, 1], BF16, tag="onesb")
            zt_sb = res.tile([P, 8], F32, tag="zt_sb")
            ztp = res.tile([P, NRT], F32, tag="ztp")
            part = res.tile([1, 1], F32, tag="part")

            mega = psum.tile([P, 4096], F32, tag="mega")

            # fp8 staging: host pre-chunked layouts, split across the
            # HWDGE (sync) and SWDGE (gpsimd) queues so ht8 (1MB, gates
            # the hp2T projection) lands as early as possible
            nc.sync.dma_start(out=ht8_sb[:, 0:4 * RPC],
                              in_=ht_ext[:, 0:4 * RPC])
            nc.gpsimd.dma_start(out=ht8_sb[:, 4 * RPC:8 * RPC],
                                in_=ht_ext[:, 4 * RPC:8 * RPC])
            nc.sync.dma_start(out=wp2_8[:], in_=wp2_ext[:, :])
            nc.sync.dma_start(out=wp1_8[:], in_=wp1_ext[:, :])
            nc.sync.dma_start(out=tf_sb[:], in_=tf_ext[:, :])

            nc.vector.memset(zbigA[:], 0.0)
            nc.vector.memset(ones[:], 1.0)
            nc.vector.memset(onesb[:], 1.0)
            nc.vector.memset(ztp[:], 0.0)
            warm = res.tile([1, 1], F32, tag="warm")
            nc.scalar.activation(warm[0:1, 0:1], ones[0:1, 0:1], ACTF.Exp)
            # PE warmup: ~40 tiny matmuls on a garbage tile unthrottle the
            # HAM clock gate (~3.4us of activity) before the real stream
            junk = res.tile([P, 4 * P], FP8, tag="junk")
            nc.vector.memset(junk[:], 0.0)
            for wi_ in range(40):
                nc.tensor.matmul(
                    out=mega[:, 3072 + (wi_ % 2) * P:
                             3072 + (wi_ % 2) * P + P],
                    lhsT=junk[:, 0:P], rhs=junk[:, (wi_ % 3) * P:
                                                (wi_ % 3) * P + P],
                    start=True, stop=True)

            # ---------------- masks and in-tier indices ----------------
            nc.vector.tensor_scalar(out=ge1[:], in0=tf_sb[:], scalar1=float(V0),
                                    scalar2=None, op0=ALU.is_ge)
            nc.vector.tensor_scalar(out=ge2[:], in0=tf_sb[:],
                                    scalar1=float(V0 + V1), scalar2=None,
                                    op0=ALU.is_ge)
            nc.vector.tensor_scalar(out=idxf[0][:], in0=tf_sb[:],
                                    scalar1=float(V0 - 1), scalar2=None,
                                    op0=ALU.min)
            nc.vector.tensor_scalar(out=idxf[1][:], in0=tf_sb[:],
                                    scalar1=-float(V0), scalar2=0.0,
                                    op0=ALU.add, op1=ALU.max)
            nc.vector.tensor_scalar(out=idxf[1][:], in0=idxf[1][:],
                                    scalar1=float(V1 - 1), scalar2=None,
                                    op0=ALU.min)
            nc.vector.tensor_scalar(out=idxf[2][:], in0=tf_sb[:],
                                    scalar1=-float(V0 + V1), scalar2=0.0,
                                    op0=ALU.add, op1=ALU.max)
            nc.vector.tensor_scalar(out=idxf[2][:], in0=idxf[2][:],
                                    scalar1=float(V2 - 1), scalar2=None,
                                    op0=ALU.min)
            for t in range(3):
                nc.vector.tensor_copy(out=idxi[t][:], in_=idxf[t][:])

            ht8v = ht8_sb[:].rearrange("p (k r) -> p k r", k=8)
            wp18v = wp1_8[:].rearrange("p (k c) -> p k c", k=8)
            wp28v = wp2_8[:].rearrange("p (k c) -> p k c", k=8)
            hp1Tv = hp1T_sb[:].rearrange("p (k r) -> p k r", k=2)

            # round-slot rotation over the mega tile
            slot_i = [0]

            def next_slot():
                off, w = CYCLE[slot_i[0] % 3]
                slot_i[0] += 1
                return off, w

            def next_slot_min(minw):
                while True:
                    off, w = next_slot()
                    if w >= minw:
                        return off, w

            # greedy engine-balance state (ns)
            eng_t = {"A": 0.0, "V": 12000.0}
            zcols = [0] * NRT

            # ---------------- hp2T projection (runway prerequisite) -------
            base, cap = next_slot_min(RPC)
            for g in range(2):
                for pr in range(4):
                    nc.tensor.matmul(
                        out=mega[:, base + g * GW: base + (g + 1) * GW],
                        lhsT=wp28v[:, 2 * pr: 2 * pr + 2, 0:P],
                        rhs=ht8v[:, 2 * pr: 2 * pr + 2, g * GW:(g + 1) * GW],
                        start=(pr == 0), stop=(pr == 3), perf_mode=DR)
            nc.vector.tensor_copy(out=hp2T_sb[:],
                                  in_=mega[:, base: base + RPC])
            eng_t["V"] += (RPC + 120) / 0.96 + 60

            # ---------------- main stream ----------------
            # tier -> (V, Kchunks, w_ext, wpool, doublerow)
            tiers = {
                0: (V0, 8, w0_ext, w0pool, True),
                1: (V1, 2, w1_ext, w1pool, True),
                2: (V2, 1, w2_ext, w2pool, False),
            }
            gather_src = [wt0_ext, wt1_ext, wt2_ext]
            gdim = [D, PD1, PD2]
            gmax = [V0 - 1, V1 - 1, V2 - 1]
            st_wtile = {}

            def ensure_st(tier, st):
                if (tier, st) in st_wtile:
                    return
                V, K, w_ext, wpool, dr = tiers[tier]
                w = min(ST, V - st * ST)
                wtile = wpool.tile([P, K * ST], FP8,
                                   tag=f"w{tier}", name=f"w{tier}")
                for k in range(K):
                    nc.gpsimd.dma_start(
                        out=wtile[:, k * ST: k * ST + w],
                        in_=w_ext[k * P:(k + 1) * P, st * ST: st * ST + w])
                st_wtile[(tier, st)] = wtile

            def st_groups(tier, st):
                V = tiers[tier][0]
                w = min(ST, V - st * ST)
                return [(tier, st, g, min(GW, w - g * GW))
                        for g in range(_ceil_div(w, GW))]

            def emit_round(groups, rt, useV):
                base, cap = next_slot()
                flush_ones()
                off = 0
                for (tier, st, g, gw) in groups:
                    V, K, w_ext, wpool, dr = tiers[tier]
                    wtile = st_wtile[(tier, st)]
                    dst = mega[:, base + off: base + off + gw]
                    if dr:
                        wv = wtile[:].rearrange("p (k c) -> p k c", k=K)
                        lv = ht8v if tier == 0 else hp1Tv
                        for pr in range(K // 2):
                            nc.tensor.matmul(
                                out=dst,
                                lhsT=lv[:, 2 * pr: 2 * pr + 2,
                                        rt * P: rt * P + P],
                                rhs=wv[:, 2 * pr: 2 * pr + 2,
                                       g * GW: g * GW + gw],
                                start=(pr == 0), stop=(pr == K // 2 - 1),
                                perf_mode=DR)
                    else:
                        nc.tensor.matmul(
                            out=dst,
                            lhsT=hp2T_sb[:, rt * P: rt * P + P],
                            rhs=wtile[:, g * GW: g * GW + gw],
                            start=True, stop=True)
                    off += gw
                src = mega[:, base: base + off]
                if useV:
                    tier = groups[0][0]
                    e16 = e16pool.tile([P, 1536], I16, tag="e16")
                    nc.vector.tensor_scalar(
                        out=e16[:, :off], in0=src,
                        scalar1=EXP_C1, scalar2=EXP_C2[tier],
                        op0=ALU.mult, op1=ALU.add)
                    va = vacc[:, rt * VW: rt * VW + off]
                    if rt in G_RTS:
                        nc.gpsimd.tensor_tensor(
                            out=va, in0=va, in1=e16[:, :off].bitcast(BF16),
                            op=ALU.add)
                        eng_t["V"] += (off + 120) / 0.96 + 60.0
                    else:
                        nc.vector.tensor_tensor(
                            out=va, in0=va, in1=e16[:, :off].bitcast(BF16),
                            op=ALU.add)
                        eng_t["V"] += _cost_dve(off)
                else:
                    zcol = rt * ZC + zcols[rt]
                    zcols[rt] += 1
                    ex = expool.tile([P, 1536], BF16, tag="ex")
                    nc.scalar.activation(
                        ex[:, :off], src, ACTF.Exp,
                        accum_out=zbigA[:, zcol: zcol + 1])
                    eng_t["A"] += _cost_act(off)

            last_eng = ["A"]

            # transposed tier-2 units: (st, vslice, rowblock)
            trans_q = [(st, v, b) for st in T_STS for v in range(16)
                       for b in (0, 1)]
            n_trans = len(trans_q)
            trans_state = {"emitted": 0, "first": True}
            pending_ones = []

            def flush_ones(all_=False):
                while len(pending_ones) >= (1 if all_ else 4):
                    for (e16t, uoff, blk) in pending_ones.pop(0):
                        trans_state["emitted"] += 1
                        nc.tensor.matmul(
                            out=mega[blk * 32: blk * 32 + 1,
                                     ZBASE: ZBASE + GW],
                            lhsT=onesb[:, 0:1],
                            rhs=e16t[:, uoff: uoff + GW].bitcast(BF16),
                            start=trans_state["first"],
                            stop=(trans_state["emitted"] == n_trans),
                            skip_group_check=True)
                        trans_state["first"] = False

            def emit_trans_round():
                cap_next = CYCLE[slot_i[0] % 3][1]
                avail = trans_avail()
                n = min(cap_next // GW, len(avail))
                if n == 0:
                    return False
                units = avail[:n]
                for u in units:
                    trans_q.remove(u)
                base, cap = next_slot()
                flush_ones()
                for u, (st, v, b) in enumerate(units):
                    wtile = st_wtile[(2, st)]
                    nc.tensor.matmul(
                        out=mega[:, base + u * GW: base + (u + 1) * GW],
                        lhsT=wtile[:, v * P:(v + 1) * P],
                        rhs=hp2T_sb[:, b * GW:(b + 1) * GW],
                        start=True, stop=True)
                off = len(units) * GW
                e16 = te16pool.tile([P, 1536], I16, tag="te16")
                nc.vector.tensor_scalar(
                    out=e16[:, :off], in0=mega[:, base: base + off],
                    scalar1=EXP_C1, scalar2=EXP_C2[2],
                    op0=ALU.mult, op1=ALU.add)
                eng_t["V"] += (off + 120) / 0.96 + 60.0
                pending_ones.append(
                    [(e16, u * GW, b) for u, (st, v, b) in enumerate(units)])
                return True

            def trans_avail():
                return [u for u in trans_q if (2, u[0]) in st_wtile]

            def plan_emit(tier_lists, rt):
                # per-tier queues; rounds draw via largest-remaining-fraction
                qs = [list(l) for l in tier_lists if l]
                tot = [len(q) for q in qs]
                while any(qs):
                    cap = CYCLE[slot_i[0] % 3][1]
                    if cap == GW and emit_trans_round():
                        continue
                    nfit = cap // GW
                    # tier-pure candidate for a V round: tier with the most
                    # remaining groups
                    vi = max(range(len(qs)), key=lambda j: len(qs[j]))
                    vgroups = qs[vi][:nfit]
                    # A-round candidate: Bresenham across tiers
                    apick = []
                    idx = [0] * len(qs)
                    for _ in range(nfit):
                        best, bj = -1.0, -1
                        for j, q in enumerate(qs):
                            rema = len(q) - idx[j]
                            if rema > 0 and rema / tot[j] > best:
                                best, bj = rema / tot[j], j
                        if bj < 0:
                            break
                        apick.append((bj, idx[bj]))
                        idx[bj] += 1
                    agroups = [qs[j][k] for (j, k) in apick]
                    wV = sum(g[3] for g in vgroups)
                    wA = sum(g[3] for g in agroups)
                    # alternation bias against same-engine streaks
                    bias = 250.0 if last_eng[0] == "A" else -250.0
                    useV = bool(vgroups) and (
                        eng_t["V"] + _cost_dve(wV) - bias <
                        eng_t["A"] + _cost_act(wA))
                    if useV:
                        qs[vi] = qs[vi][nfit:]
                        emit_round(vgroups, rt, True)
                        last_eng[0] = "V"
                    else:
                        for (j, k) in sorted(apick, reverse=True):
                            qs[j].pop(k)
                        emit_round(agroups, rt, False)
                        last_eng[0] = "A"

            def emit_rt_final(rt):
                # row-tile Z reduction, emitted as soon as rt's stream ends
                nc.vector.tensor_reduce(
                    out=zredA[:, rt:rt + 1],
                    in_=zbigA[:, rt * ZC:(rt + 1) * ZC],
                    axis=mybir.AxisListType.X, op=ALU.add)
                eng_t["V"] += (ZC + 58) / 0.96 + 60
                cA = (VW + 224) / 1.2 + 181
                cV = (VW + 58) / 0.96
                if eng_t["A"] + cA < eng_t["V"] + cV:
                    ex = expool.tile([P, 1536], BF16, tag="ex")
                    nc.scalar.activation(
                        ex[:, :VW], vacc[:, rt * VW:(rt + 1) * VW],
                        ACTF.Identity, accum_out=zredV[:, rt:rt + 1])
                    eng_t["A"] += cA
                else:
                    nc.vector.tensor_reduce(
                        out=zredV[:, rt:rt + 1],
                        in_=vacc[:, rt * VW:(rt + 1) * VW],
                        axis=mybir.AxisListType.X, op=ALU.add)
                    eng_t["V"] += cV

            def emit_rows_proj(rt, t):
                # DR rows-orientation projection feeding the target dot
                pd = PD1 if t == 1 else PD2
                wv = wp18v if t == 1 else wp28v
                dstt = hp1r_sb if t == 1 else hp2r_sb
                base, cap = next_slot()
                for pr in range(4):
                    nc.tensor.matmul(
                        out=mega[:, base: base + pd],
                        lhsT=ht8v[:, 2 * pr: 2 * pr + 2,
                                  rt * P: rt * P + P],
                        rhs=wv[:, 2 * pr: 2 * pr + 2, 0:pd],
                        start=(pr == 0), stop=(pr == 3), perf_mode=DR)
                nc.vector.tensor_copy(
                    out=dstt[:, rt * pd:(rt + 1) * pd],
                    in_=mega[:, base: base + pd])
                eng_t["V"] += (pd + 120) / 0.96 + 60

            def emit_gather_dot(i):
                rt, t = divmod(i, 3)
                if t == 0:
                    hr_t = hrpool.tile([P, D], BF16, tag="hrt", name="hrt")
                    nc.sync.dma_start(out=hr_t[:],
                                      in_=hr_ext[rt * P:(rt + 1) * P, :])
                    feat_ap = hr_t[:]
                elif t == 1:
                    emit_rows_proj(rt, 1)
                    feat_ap = hp1r_sb[:, rt * PD1:(rt + 1) * PD1]
                else:
                    emit_rows_proj(rt, 2)
                    feat_ap = hp2r_sb[:, rt * PD2:(rt + 1) * PD2]
                g = gpool.tile([P, gdim[t]], BF16, tag=f"g{t}", name=f"g{t}")
                nc.gpsimd.indirect_dma_start(
                    out=g[:], out_offset=None,
                    in_=gather_src[t][:, :],
                    in_offset=IndirectOffsetOnAxis(
                        ap=idxi[t][:, rt:rt + 1], axis=0),
                    bounds_check=gmax[t], oob_is_err=False)
                prod = prodpool.tile([P, D], BF16, tag="prod")
                nc.vector.scalar_tensor_tensor(
                    out=prod[:, :gdim[t]],
                    in0=feat_ap, scalar=1.0, in1=g[:],
                    op0=ALU.mult, op1=ALU.mult,
                    accum_out=tl[t][:, rt:rt + 1])
                eng_t["V"] += (gdim[t] / 2 + 58) / 0.96 + 60

            def interleave(lists):
                # Bresenham-style proportional merge of per-tier group lists
                out = []
                idx = [0] * len(lists)
                tot = [len(l) for l in lists]
                n = sum(tot)
                for _ in range(n):
                    best, bi = -1.0, 0
                    for j, l in enumerate(lists):
                        if idx[j] < tot[j]:
                            frac = (tot[j] - idx[j]) / tot[j]
                            if frac > best:
                                best, bi = frac, j
                    out.append(lists[bi][idx[bi]])
                    idx[bi] += 1
                return out

            gi = 0
            for wi, (a_st, b_sts, c_sts) in enumerate(WINDOWS):
                for st in c_sts:
                    ensure_st(2, st)
                ensure_st(0, a_st)
                for st in b_sts:
                    ensure_st(1, st)
                As = st_groups(0, a_st)
                Bs = [g for st in b_sts for g in st_groups(1, st)]
                Cs = [g for st in c_sts if st not in T_STS
                      for g in st_groups(2, st)]
                if wi == 0:
                    # runway: tier2 rounds only while w0/w1 land; vacc
                    # slices are zeroed here (V is otherwise idle early)
                    for rt in range(NRT):
                        if rt in G_RTS:
                            nc.gpsimd.memset(
                                vacc[:, rt * VW:(rt + 1) * VW], 0.0)
                        else:
                            nc.vector.memset(
                                vacc[:, rt * VW:(rt + 1) * VW], 0.0)
                        plan_emit([Cs[0:8]], rt)
                    # hp1T projection: needed by the first B rounds
                    for m in range(2):
                        base, cap = next_slot_min(RPC)
                        for g in range(2):
                            for pr in range(4):
                                nc.tensor.matmul(
                                    out=mega[:, base + g * GW:
                                             base + (g + 1) * GW],
                                    lhsT=wp18v[:, 2 * pr: 2 * pr + 2,
                                               m * P:(m + 1) * P],
                                    rhs=ht8v[:, 2 * pr: 2 * pr + 2,
                                             g * GW:(g + 1) * GW],
                                    start=(pr == 0), stop=(pr == 3),
                                    perf_mode=DR)
                        nc.vector.tensor_copy(
                            out=hp1T_sb[:, m * RPC:(m + 1) * RPC],
                            in_=mega[:, base: base + RPC])
                        eng_t["V"] += (RPC + 120) / 0.96 + 60
                    for rt in range(NRT):
                        plan_emit([Cs[8:], As, Bs], rt)
                    continue
                for rt in range(NRT):
                    plan_emit([As, Bs, Cs], rt)
                    for _ in range(2):
                        if trans_avail() and eng_t["V"] < eng_t["A"]:
                            emit_trans_round()
                    if gi < 3 * NRT:
                        emit_gather_dot(gi)
                        gi += 1
                    if wi == 3:
                        emit_rt_final(rt)
            while trans_q:
                if not emit_trans_round():
                    raise RuntimeError(f"stuck: {len(trans_q)} units left")
            flush_ones(all_=True)
            while gi < 3 * NRT:
                emit_gather_dot(gi)
                gi += 1
            # transposed-Z finale: PSUM row-vectors -> DRAM -> per-partition
            ztmp = res.tile([P, GW], F32, tag="ztmp")
            zdram = nc.dram_tensor("zscratch", [2, GW], F32, kind="Internal")
            for b in (0, 1):
                nc.vector.tensor_copy(
                    out=ztmp[b * 32: b * 32 + 1, :],
                    in_=mega[b * 32: b * 32 + 1, ZBASE: ZBASE + GW])
                nc.sync.dma_start(
                    out=zdram[b: b + 1, :],
                    in_=ztmp[b * 32: b * 32 + 1, :])
            nc.sync.dma_start(
                out=ztp[:, :],
                in_=zdram[:, :].rearrange("b (r p) -> p (b r)", p=P))

            # ---------------- final reduction ----------------
            # zred = zredA + zredV + d1 (d1 holds the ScalarE-reduced
            # second vacc half where that path was taken)
            nc.vector.tensor_tensor(out=zred[:], in0=zredA[:], in1=zredV[:],
                                    op=ALU.add)
            nc.vector.tensor_tensor(out=zred[:], in0=zred[:], in1=ztp[:],
                                    op=ALU.add)
            nc.scalar.activation(logz[:], zred[:], ACTF.Ln)
            # loss8 = logz - (tl0 + ge1*(tl1-tl0) + ge2*(tl2-tl1))
            nc.vector.tensor_tensor(out=d1[:], in0=tl[1][:], in1=tl[0][:],
                                    op=ALU.subtract)
            nc.vector.tensor_tensor(out=d2[:], in0=tl[2][:], in1=tl[1][:],
                                    op=ALU.subtract)
            nc.vector.tensor_tensor(out=d1[:], in0=d1[:], in1=ge1[:],
                                    op=ALU.mult)
            nc.vector.tensor_tensor(out=d2[:], in0=d2[:], in1=ge2[:],
                                    op=ALU.mult)
            nc.vector.tensor_tensor(out=loss8[:], in0=logz[:], in1=tl[0][:],
                                    op=ALU.subtract)
            nc.vector.tensor_tensor(out=loss8[:], in0=loss8[:], in1=d1[:],
                                    op=ALU.subtract)
            nc.vector.tensor_tensor(out=loss8[:], in0=loss8[:], in1=d2[:],
                                    op=ALU.subtract)
            nc.vector.tensor_reduce(out=lossv[:], in_=loss8[:],
                                    axis=mybir.AxisListType.X, op=ALU.add)
            base, cap = next_slot()
            nc.tensor.matmul(out=mega[0:1, base:base + 1], lhsT=lossv[:],
                             rhs=ones[:], start=True, stop=True)
            nc.scalar.mul(part[0:1, 0:1], mega[0:1, base:base + 1],
                          1.0 / float(B_T))
            nc.sync.dma_start(out=out_ext[:, :], in_=part[:])

    nc.compile()
    return nc


def _get_nc():
    global _NC_CACHE
    if _NC_CACHE is None:
        _NC_CACHE = _build_graph()
    return _NC_CACHE


def _make_in_maps(h, targets, W_head0, W_proj1, W_head1, W_proj2, W_head2):
    FP8NP = ml_dtypes.float8_e4m3
    BF16NP = ml_dtypes.bfloat16
    h = np.ascontiguousarray(np.asarray(h, dtype=np.float32)).reshape(B_T, D)
    t = np.asarray(targets).reshape(-1).astype(np.float32)
    w0 = np.asarray(W_head0, dtype=np.float32)
    w1 = np.asarray(W_head1, dtype=np.float32)
    w2 = np.asarray(W_head2, dtype=np.float32)
    wp1 = np.asarray(W_proj1, dtype=np.float32)
    wp2 = np.asarray(W_proj2, dtype=np.float32)
    w0_8 = np.ascontiguousarray(w0.astype(FP8NP))
    w1_8 = np.ascontiguousarray(w1.astype(FP8NP))
    w2_8 = np.ascontiguousarray(w2.astype(FP8NP))
    wp1_c = np.ascontiguousarray(
        wp1.astype(FP8NP).reshape(8, P, PD1).transpose(1, 0, 2).reshape(
            P, 8 * PD1))
    wp2_c = np.ascontiguousarray(
        wp2.astype(FP8NP).reshape(8, P, PD2).transpose(1, 0, 2).reshape(
            P, 8 * PD2))
    wt0 = np.ascontiguousarray(w0.T.astype(BF16NP))
    wt1 = np.ascontiguousarray(w1.T.astype(BF16NP))
    wt2 = np.ascontiguousarray(w2.T.astype(BF16NP))

    in_maps = []
    for c in range(N_CORES):
        hc = h[c * RPC:(c + 1) * RPC]
        tc_ = t[c * RPC:(c + 1) * RPC]
        ht8 = hc.T.astype(FP8NP).reshape(8, P, RPC).transpose(1, 0, 2)
        in_maps.append({
            "ht": np.ascontiguousarray(ht8.reshape(P, 8 * RPC)),
            "hr": np.ascontiguousarray(hc.astype(BF16NP)),
            "tf": np.ascontiguousarray(tc_.reshape(NRT, P).T),
            "wp1": wp1_c, "wp2": wp2_c,
            "w0": w0_8, "w1": w1_8, "w2": w2_8,
            "wt0": wt0, "wt1": wt1, "wt2": wt2,
        })
    return in_maps


def _finalize(results):
    total = sum(float(results[c]["out"][0, 0]) for c in range(N_CORES))
    return np.float32(total)


def kernel(h, targets, token_to_tier, token_to_idx,
           W_head0, W_proj1, W_head1, W_proj2, W_head2):
    in_maps = _make_in_maps(h, targets, W_head0, W_proj1, W_head1,
                            W_proj2, W_head2)
    nc = _get_nc()
    res = run_bass_kernel_spmd(nc, in_maps, core_ids=list(range(N_CORES)))
    return _finalize(res.results)


# revision 33
# speedup vs baseline: 1.2309x; 1.1183x over previous
"""Adaptive LM head (3-tier chunked softmax cross-entropy) on 8 TRN2 NeuronCores.

Strategy: data-parallel over B_T = 8192 rows (1024 rows/core; weights
replicated, pre-cast to fp8 on the host). The kernel is bound by draining
softmax logits out of PSUM (exp + row-sum of 51.5M elements/core), so that
work is split across both PSUM-capable engines:
  - ScalarE: true exp via one merged activation(Exp, accum_out=...) per
    PSUM round (up to 1536 wide, amortizing the ~400ns instruction+
    accumulator-read overhead).
  - VectorE: Schraudolph fast-exp on tier-pure rounds: one tensor_scalar
    computes int16(round(l*2^7/ln2 + C2_tier)) == the bit pattern of
    bf16(e^l) (C2 calibrated per tier to zero the Z bias), then a bf16
    2x-mode tensor_tensor adds the bits-view into a per-row-tile
    accumulator.
  - TensorE: fp8 DoubleRow matmuls (tiers 0/1) and fp8 matmuls (tier 2)
    into a manually rotated [128,4096] f32 PSUM mega-tile; rounds are
    bank-aligned [1536,1536,1024] so three rounds are in flight.
  - target logit = dot(feature_row, W[:, target]) in bf16: indirect-DMA
    gather of transposed-weight rows + scalar_tensor_tensor multiply-
    reduce, spread through the main stream.
  - per-core partial loss (sum_rows(log Z - target_logit)/8192) is the
    output; the host sums the 8 partials.
"""

import numpy as np
import ml_dtypes

from concourse import bacc, bass, mybir
from concourse.bass import IndirectOffsetOnAxis
from concourse.bass_utils import run_bass_kernel_spmd
from concourse.tile import TileContext

F32 = mybir.dt.float32
BF16 = mybir.dt.bfloat16
I32 = mybir.dt.int32
I16 = mybir.dt.int16
FP8 = mybir.dt.float8e4
DR = mybir.MatmulPerfMode.DoubleRow
ALU = mybir.AluOpType
ACTF = mybir.ActivationFunctionType

P = 128
D = 1024
N_CORES = 8
RPC = 1024          # rows per core
NRT = RPC // P      # row tiles per core = 8
ST = 2048           # vocab super-tile width (DMA granularity)
GW = 512            # psum group width (one f32 PSUM bank)
V0, V1, V2 = 8192, 16384, 25681
PD1, PD2 = 256, 128
B_T = 8192
ZC = 64             # zbigA columns per row-tile
VW = 1536           # vacc columns per row-tile (max V-round width)

# PSUM mega-tile round rotation: bank-aligned offsets/widths.
CYCLE = ((0, 1536), (1536, 1536), (3072, 1024))

# Schraudolph bf16 fast-exp: bits16(e^x) ~= round(x*C1 + C2_t).
# C2 calibrated per tier against the tier's logit sigma so the mean
# multiplicative bias of sum(exp) is ~0 (sigma: t0~0.64, t1~0.21, t2~0.15).
EXP_C1 = float(2.0 ** 7 / np.log(2.0))
EXP_C2 = (16256.0 - 7.25, 16256.0 - 6.5, 16256.0 - 5.5)

# DMA windows: (tier0 st, [tier1 sts], [tier2 sts]) of 2048-col supertiles.
WINDOWS = [
    (0, [0, 1], [0, 1, 2]),
    (1, [2, 3], [3, 4, 5]),
    (2, [4, 5], [6, 7, 8]),
    (3, [6, 7], [9, 10, 11, 12]),
]

G_RTS = ()          # row-tiles whose fast-exp accumulation runs on GpSimd

_NC_CACHE = None


def _ceil_div(a, b):
    return (a + b - 1) // b


def _cost_act(w):
    # merged ACTIVATE + ACTIVATION_READ_ACCUMULATOR + dispatch
    return (w + 222) / 1.2 + 181.0 + 30.0


def _cost_dve(w):
    # pass1 (1x from PSUM) + bf16 TT-add at 2x + dispatch
    return (w + 120) / 0.96 + (w / 2 + 58) / 0.96 + 120.0


def _build_graph():
    nc = bacc.Bacc("TRN2", target_bir_lowering=False, debug=False,
                   num_devices=N_CORES)

    ht_ext = nc.declare_dram_parameter("ht", [P, 8 * RPC], FP8, isOutput=False)
    hr_ext = nc.declare_dram_parameter("hr", [RPC, D], BF16, isOutput=False)
    tf_ext = nc.declare_dram_parameter("tf", [P, NRT], F32, isOutput=False)
    wp1_ext = nc.declare_dram_parameter("wp1", [P, 8 * PD1], FP8, isOutput=False)
    wp2_ext = nc.declare_dram_parameter("wp2", [P, 8 * PD2], FP8, isOutput=False)
    w0_ext = nc.declare_dram_parameter("w0", [D, V0], FP8, isOutput=False)
    w1_ext = nc.declare_dram_parameter("w1", [PD1, V1], FP8, isOutput=False)
    w2_ext = nc.declare_dram_parameter("w2", [PD2, V2], FP8, isOutput=False)
    wt0_ext = nc.declare_dram_parameter("wt0", [V0, D], BF16, isOutput=False)
    wt1_ext = nc.declare_dram_parameter("wt1", [V1, PD1], BF16, isOutput=False)
    wt2_ext = nc.declare_dram_parameter("wt2", [V2, PD2], BF16, isOutput=False)
    out_ext = nc.declare_dram_parameter("out", [1, 1], F32, isOutput=True)

    with TileContext(nc) as tc:
        with (
            tc.tile_pool(name="res", bufs=1) as res,
            tc.tile_pool(name="w0pool", bufs=2) as w0pool,
            tc.tile_pool(name="w1pool", bufs=4) as w1pool,
            tc.tile_pool(name="w2pool", bufs=6) as w2pool,
            tc.tile_pool(name="hrpool", bufs=2) as hrpool,
            tc.tile_pool(name="expool", bufs=3) as expool,
            tc.tile_pool(name="e16pool", bufs=6) as e16pool,
            tc.tile_pool(name="gpool", bufs=2) as gpool,
            tc.tile_pool(name="prodpool", bufs=2) as prodpool,
            tc.tile_pool(name="psum", bufs=1, space="PSUM") as psum,
        ):
            # ---------------- resident tiles ----------------
            ht8_sb = res.tile([P, 8 * RPC], FP8, tag="ht8")
            wp1_8 = res.tile([P, 8 * PD1], FP8, tag="wp18")
            wp2_8 = res.tile([P, 8 * PD2], FP8, tag="wp28")
            hp1T_sb = res.tile([P, 2 * RPC], FP8, tag="hp1T")
            hp2T_sb = res.tile([P, 1 * RPC], FP8, tag="hp2T")
            hp1r_sb = res.tile([P, NRT * PD1], BF16, tag="hp1r")
            hp2r_sb = res.tile([P, NRT * PD2], BF16, tag="hp2r")
            tf_sb = res.tile([P, NRT], F32, tag="tf")
            ge1 = res.tile([P, NRT], F32, tag="ge1")
            ge2 = res.tile([P, NRT], F32, tag="ge2")
            idxf = [res.tile([P, NRT], F32, tag=f"idxf{t}", name=f"idxf{t}")
                    for t in range(3)]
            idxi = [res.tile([P, NRT], I32, tag=f"idxi{t}", name=f"idxi{t}")
                    for t in range(3)]
            tl = [res.tile([P, NRT], F32, tag=f"tl{t}", name=f"tl{t}")
                  for t in range(3)]
            zbigA = res.tile([P, NRT * ZC], F32, tag="zbigA")
            vacc = res.tile([P, NRT * VW], BF16, tag="vacc")
            zredA = res.tile([P, NRT], F32, tag="zredA")
            zredV = res.tile([P, NRT], F32, tag="zredV")
            zred = res.tile([P, NRT], F32, tag="zred")
            logz = res.tile([P, NRT], F32, tag="logz")
            d1 = res.tile([P, NRT], F32, tag="d1")
            d2 = res.tile([P, NRT], F32, tag="d2")
            loss8 = res.tile([P, NRT], F32, tag="loss8")
            lossv = res.tile([P, 1], F32, tag="lossv")
            ones = res.tile([P, 1], F32, tag="ones")
            part = res.tile([1, 1], F32, tag="part")

            mega = psum.tile([P, 4096], F32, tag="mega")

            # fp8 staging: host pre-chunked layouts, split across the
            # HWDGE (sync) and SWDGE (gpsimd) queues so ht8 (1MB, gates
            # the hp2T projection) lands as early as possible
            nc.sync.dma_start(out=ht8_sb[:, 0:4 * RPC],
                              in_=ht_ext[:, 0:4 * RPC])
            nc.gpsimd.dma_start(out=ht8_sb[:, 4 * RPC:8 * RPC],
                                in_=ht_ext[:, 4 * RPC:8 * RPC])
            nc.sync.dma_start(out=wp2_8[:], in_=wp2_ext[:, :])
            nc.sync.dma_start(out=wp1_8[:], in_=wp1_ext[:, :])
            nc.sync.dma_start(out=tf_sb[:], in_=tf_ext[:, :])

            nc.vector.memset(zbigA[:], 0.0)
            nc.vector.memset(ones[:], 1.0)
            warm = res.tile([1, 1], F32, tag="warm")
            nc.scalar.activation(warm[0:1, 0:1], ones[0:1, 0:1], ACTF.Exp)
            # PE warmup: ~40 tiny matmuls on a garbage tile unthrottle the
            # HAM clock gate (~3.4us of activity) before the real stream
            junk = res.tile([P, 4 * P], FP8, tag="junk")
            nc.vector.memset(junk[:], 0.0)
            for wi_ in range(40):
                nc.tensor.matmul(
                    out=mega[:, 3072 + (wi_ % 2) * GW:
                             3072 + (wi_ % 2) * GW + P],
                    lhsT=junk[:, 0:P], rhs=junk[:, (wi_ % 3) * P:
                                                (wi_ % 3) * P + P],
                    start=True, stop=True)

            # ---------------- masks and in-tier indices ----------------
            nc.vector.tensor_scalar(out=ge1[:], in0=tf_sb[:], scalar1=float(V0),
                                    scalar2=None, op0=ALU.is_ge)
            nc.vector.tensor_scalar(out=ge2[:], in0=tf_sb[:],
                                    scalar1=float(V0 + V1), scalar2=None,
                                    op0=ALU.is_ge)
            nc.vector.tensor_scalar(out=idxf[0][:], in0=tf_sb[:],
                                    scalar1=float(V0 - 1), scalar2=None,
                                    op0=ALU.min)
            nc.vector.tensor_scalar(out=idxf[1][:], in0=tf_sb[:],
                                    scalar1=-float(V0), scalar2=0.0,
                                    op0=ALU.add, op1=ALU.max)
            nc.vector.tensor_scalar(out=idxf[1][:], in0=idxf[1][:],
                                    scalar1=float(V1 - 1), scalar2=None,
                                    op0=ALU.min)
            nc.vector.tensor_scalar(out=idxf[2][:], in0=tf_sb[:],
                                    scalar1=-float(V0 + V1), scalar2=0.0,
                                    op0=ALU.add, op1=ALU.max)
            nc.vector.tensor_scalar(out=idxf[2][:], in0=idxf[2][:],
                                    scalar1=float(V2 - 1), scalar2=None,
                                    op0=ALU.min)
            for t in range(3):
                nc.vector.tensor_copy(out=idxi[t][:], in_=idxf[t][:])

            ht8v = ht8_sb[:].rearrange("p (k r) -> p k r", k=8)
            wp18v = wp1_8[:].rearrange("p (k c) -> p k c", k=8)
            wp28v = wp2_8[:].rearrange("p (k c) -> p k c", k=8)
            hp1Tv = hp1T_sb[:].rearrange("p (k r) -> p k r", k=2)

            # round-slot rotation over the mega tile
            slot_i = [0]

            def next_slot():
                off, w = CYCLE[slot_i[0] % 3]
                slot_i[0] += 1
                return off, w

            # greedy engine-balance state (ns)
            eng_t = {"A": 0.0, "V": 12000.0}
            zcols = [0] * NRT

            # ---------------- hp2T projection (runway prerequisite) -------
            base, cap = next_slot()
            for g in range(2):
                for pr in range(4):
                    nc.tensor.matmul(
                        out=mega[:, base + g * GW: base + (g + 1) * GW],
                        lhsT=wp28v[:, 2 * pr: 2 * pr + 2, 0:P],
                        rhs=ht8v[:, 2 * pr: 2 * pr + 2, g * GW:(g + 1) * GW],
                        start=(pr == 0), stop=(pr == 3), perf_mode=DR)
            nc.vector.tensor_copy(out=hp2T_sb[:],
                                  in_=mega[:, base: base + RPC])
            eng_t["V"] += (RPC + 120) / 0.96 + 60

            # ---------------- main stream ----------------
            # tier -> (V, Kchunks, w_ext, wpool, doublerow)
            tiers = {
                0: (V0, 8, w0_ext, w0pool, True),
                1: (V1, 2, w1_ext, w1pool, True),
                2: (V2, 1, w2_ext, w2pool, False),
            }
            gather_src = [wt0_ext, wt1_ext, wt2_ext]
            gdim = [D, PD1, PD2]
            gmax = [V0 - 1, V1 - 1, V2 - 1]
            st_wtile = {}

            def ensure_st(tier, st):
                if (tier, st) in st_wtile:
                    return
                V, K, w_ext, wpool, dr = tiers[tier]
                w = min(ST, V - st * ST)
                wtile = wpool.tile([P, K * ST], FP8,
                                   tag=f"w{tier}", name=f"w{tier}")
                for k in range(K):
                    nc.gpsimd.dma_start(
                        out=wtile[:, k * ST: k * ST + w],
                        in_=w_ext[k * P:(k + 1) * P, st * ST: st * ST + w])
                st_wtile[(tier, st)] = wtile

            def st_groups(tier, st):
                V = tiers[tier][0]
                w = min(ST, V - st * ST)
                return [(tier, st, g, min(GW, w - g * GW))
                        for g in range(_ceil_div(w, GW))]

            def emit_round(groups, rt, useV):
                base, cap = next_slot()
                off = 0
                for (tier, st, g, gw) in groups:
                    V, K, w_ext, wpool, dr = tiers[tier]
                    wtile = st_wtile[(tier, st)]
                    dst = mega[:, base + off: base + off + gw]
                    if dr:
                        wv = wtile[:].rearrange("p (k c) -> p k c", k=K)
                        lv = ht8v if tier == 0 else hp1Tv
                        for pr in range(K // 2):
                            nc.tensor.matmul(
                                out=dst,
                                lhsT=lv[:, 2 * pr: 2 * pr + 2,
                                        rt * P: rt * P + P],
                                rhs=wv[:, 2 * pr: 2 * pr + 2,
                                       g * GW: g * GW + gw],
                                start=(pr == 0), stop=(pr == K // 2 - 1),
                                perf_mode=DR)
                    else:
                        nc.tensor.matmul(
                            out=dst,
                            lhsT=hp2T_sb[:, rt * P: rt * P + P],
                            rhs=wtile[:, g * GW: g * GW + gw],
                            start=True, stop=True)
                    off += gw
                src = mega[:, base: base + off]
                if useV:
                    tier = groups[0][0]
                    e16 = e16pool.tile([P, 1536], I16, tag="e16")
                    nc.vector.tensor_scalar(
                        out=e16[:, :off], in0=src,
                        scalar1=EXP_C1, scalar2=EXP_C2[tier],
                        op0=ALU.mult, op1=ALU.add)
                    va = vacc[:, rt * VW: rt * VW + off]
                    if rt in G_RTS:
                        nc.gpsimd.tensor_tensor(
                            out=va, in0=va, in1=e16[:, :off].bitcast(BF16),
                            op=ALU.add)
                        eng_t["V"] += (off + 120) / 0.96 + 60.0
                    else:
                        nc.vector.tensor_tensor(
                            out=va, in0=va, in1=e16[:, :off].bitcast(BF16),
                            op=ALU.add)
                        eng_t["V"] += _cost_dve(off)
                else:
                    zcol = rt * ZC + zcols[rt]
                    zcols[rt] += 1
                    ex = expool.tile([P, 1536], BF16, tag="ex")
                    nc.scalar.activation(
                        ex[:, :off], src, ACTF.Exp,
                        accum_out=zbigA[:, zcol: zcol + 1])
                    eng_t["A"] += _cost_act(off)

            last_eng = ["A"]

            def plan_emit(tier_lists, rt):
                # per-tier queues; rounds draw via largest-remaining-fraction
                qs = [list(l) for l in tier_lists if l]
                tot = [len(q) for q in qs]
                while any(qs):
                    cap = CYCLE[slot_i[0] % 3][1]
                    nfit = cap // GW
                    # tier-pure candidate for a V round: tier with the most
                    # remaining groups
                    vi = max(range(len(qs)), key=lambda j: len(qs[j]))
                    vgroups = qs[vi][:nfit]
                    # A-round candidate: Bresenham across tiers
                    apick = []
                    idx = [0] * len(qs)
                    for _ in range(nfit):
                        best, bj = -1.0, -1
                        for j, q in enumerate(qs):
                            rema = len(q) - idx[j]
                            if rema > 0 and rema / tot[j] > best:
                                best, bj = rema / tot[j], j
                        if bj < 0:
                            break
                        apick.append((bj, idx[bj]))
                        idx[bj] += 1
                    agroups = [qs[j][k] for (j, k) in apick]
                    wV = sum(g[3] for g in vgroups)
                    wA = sum(g[3] for g in agroups)
                    # alternation bias against same-engine streaks
                    bias = 250.0 if last_eng[0] == "A" else -250.0
                    useV = bool(vgroups) and (
                        eng_t["V"] + _cost_dve(wV) - bias <
                        eng_t["A"] + _cost_act(wA))
                    if useV:
                        qs[vi] = qs[vi][nfit:]
                        emit_round(vgroups, rt, True)
                        last_eng[0] = "V"
                    else:
                        for (j, k) in sorted(apick, reverse=True):
                            qs[j].pop(k)
                        emit_round(agroups, rt, False)
                        last_eng[0] = "A"

            def emit_rt_final(rt):
                # row-tile Z reduction, emitted as soon as rt's stream ends
                nc.vector.tensor_reduce(
                    out=zredA[:, rt:rt + 1],
                    in_=zbigA[:, rt * ZC:(rt + 1) * ZC],
                    axis=mybir.AxisListType.X, op=ALU.add)
                eng_t["V"] += (ZC + 58) / 0.96 + 60
                cA = (VW + 224) / 1.2 + 181
                cV = (VW + 58) / 0.96
                if eng_t["A"] + cA < eng_t["V"] + cV:
                    ex = expool.tile([P, 1536], BF16, tag="ex")
                    nc.scalar.activation(
                        ex[:, :VW], vacc[:, rt * VW:(rt + 1) * VW],
                        ACTF.Identity, accum_out=zredV[:, rt:rt + 1])
                    eng_t["A"] += cA
                else:
                    nc.vector.tensor_reduce(
                        out=zredV[:, rt:rt + 1],
                        in_=vacc[:, rt * VW:(rt + 1) * VW],
                        axis=mybir.AxisListType.X, op=ALU.add)
                    eng_t["V"] += cV

            def emit_rows_proj(rt, t):
                # DR rows-orientation projection feeding the target dot
                pd = PD1 if t == 1 else PD2
                wv = wp18v if t == 1 else wp28v
                dstt = hp1r_sb if t == 1 else hp2r_sb
                base, cap = next_slot()
                for pr in range(4):
                    nc.tensor.matmul(
                        out=mega[:, base: base + pd],
                        lhsT=ht8v[:, 2 * pr: 2 * pr + 2,
                                  rt * P: rt * P + P],
                        rhs=wv[:, 2 * pr: 2 * pr + 2, 0:pd],
                        start=(pr == 0), stop=(pr == 3), perf_mode=DR)
                nc.vector.tensor_copy(
                    out=dstt[:, rt * pd:(rt + 1) * pd],
                    in_=mega[:, base: base + pd])
                eng_t["V"] += (pd + 120) / 0.96 + 60

            def emit_gather_dot(i):
                rt, t = divmod(i, 3)
                if t == 0:
                    hr_t = hrpool.tile([P, D], BF16, tag="hrt", name="hrt")
                    nc.sync.dma_start(out=hr_t[:],
                                      in_=hr_ext[rt * P:(rt + 1) * P, :])
                    feat_ap = hr_t[:]
                elif t == 1:
                    emit_rows_proj(rt, 1)
                    feat_ap = hp1r_sb[:, rt * PD1:(rt + 1) * PD1]
                else:
                    emit_rows_proj(rt, 2)
                    feat_ap = hp2r_sb[:, rt * PD2:(rt + 1) * PD2]
                g = gpool.tile([P, gdim[t]], BF16, tag=f"g{t}", name=f"g{t}")
                nc.gpsimd.indirect_dma_start(
                    out=g[:], out_offset=None,
                    in_=gather_src[t][:, :],
                    in_offset=IndirectOffsetOnAxis(
                        ap=idxi[t][:, rt:rt + 1], axis=0),
                    bounds_check=gmax[t], oob_is_err=False)
                prod = prodpool.tile([P, D], BF16, tag="prod")
                nc.vector.scalar_tensor_tensor(
                    out=prod[:, :gdim[t]],
                    in0=feat_ap, scalar=1.0, in1=g[:],
                    op0=ALU.mult, op1=ALU.mult,
                    accum_out=tl[t][:, rt:rt + 1])
                eng_t["V"] += (gdim[t] / 2 + 58) / 0.96 + 60

            def interleave(lists):
                # Bresenham-style proportional merge of per-tier group lists
                out = []
                idx = [0] * len(lists)
                tot = [len(l) for l in lists]
                n = sum(tot)
                for _ in range(n):
                    best, bi = -1.0, 0
                    for j, l in enumerate(lists):
                        if idx[j] < tot[j]:
                            frac = (tot[j] - idx[j]) / tot[j]
                            if frac > best:
                                best, bi = frac, j
                    out.append(lists[bi][idx[bi]])
                    idx[bi] += 1
                return out

            gi = 0
            for wi, (a_st, b_sts, c_sts) in enumerate(WINDOWS):
                for st in c_sts:
                    ensure_st(2, st)
                ensure_st(0, a_st)
                for st in b_sts:
                    ensure_st(1, st)
                As = st_groups(0, a_st)
                Bs = [g for st in b_sts for g in st_groups(1, st)]
                Cs = [g for st in c_sts for g in st_groups(2, st)]
                if wi == 0:
                    # runway: tier2 rounds only while w0/w1 land; vacc
                    # slices are zeroed here (V is otherwise idle early)
                    for rt in range(NRT):
                        if rt in G_RTS:
                            nc.gpsimd.memset(
                                vacc[:, rt * VW:(rt + 1) * VW], 0.0)
                        else:
                            nc.vector.memset(
                                vacc[:, rt * VW:(rt + 1) * VW], 0.0)
                        plan_emit([Cs[0:8]], rt)
                    # hp1T projection: needed by the first B rounds
                    for m in range(2):
                        base, cap = next_slot()
                        for g in range(2):
                            for pr in range(4):
                                nc.tensor.matmul(
                                    out=mega[:, base + g * GW:
                                             base + (g + 1) * GW],
                                    lhsT=wp18v[:, 2 * pr: 2 * pr + 2,
                                               m * P:(m + 1) * P],
                                    rhs=ht8v[:, 2 * pr: 2 * pr + 2,
                                             g * GW:(g + 1) * GW],
                                    start=(pr == 0), stop=(pr == 3),
                                    perf_mode=DR)
                        nc.vector.tensor_copy(
                            out=hp1T_sb[:, m * RPC:(m + 1) * RPC],
                            in_=mega[:, base: base + RPC])
                        eng_t["V"] += (RPC + 120) / 0.96 + 60
                    for rt in range(NRT):
                        plan_emit([Cs[8:], As, Bs], rt)
                    continue
                for rt in range(NRT):
                    plan_emit([As, Bs, Cs], rt)
                    if gi < 3 * NRT:
                        emit_gather_dot(gi)
                        gi += 1
                    if wi == 3:
                        emit_rt_final(rt)
            while gi < 3 * NRT:
                emit_gather_dot(gi)
                gi += 1

            # ---------------- final reduction ----------------
            # zred = zredA + zredV + d1 (d1 holds the ScalarE-reduced
            # second vacc half where that path was taken)
            nc.vector.tensor_tensor(out=zred[:], in0=zredA[:], in1=zredV[:],
                                    op=ALU.add)
            nc.scalar.activation(logz[:], zred[:], ACTF.Ln)
            # loss8 = logz - (tl0 + ge1*(tl1-tl0) + ge2*(tl2-tl1))
            nc.vector.tensor_tensor(out=d1[:], in0=tl[1][:], in1=tl[0][:],
                                    op=ALU.subtract)
            nc.vector.tensor_tensor(out=d2[:], in0=tl[2][:], in1=tl[1][:],
                                    op=ALU.subtract)
            nc.vector.tensor_tensor(out=d1[:], in0=d1[:], in1=ge1[:],
                                    op=ALU.mult)
            nc.vector.tensor_tensor(out=d2[:], in0=d2[:], in1=ge2[:],
                                    op=ALU.mult)
            nc.vector.tensor_tensor(out=loss8[:], in0=logz[:], in1=tl[0][:],
                                    op=ALU.subtract)
            nc.vector.tensor_tensor(out=loss8[:], in0=loss8[:], in1=d1[:],
                                    op=ALU.subtract)
            nc.vector.tensor_tensor(out=loss8[:], in0=loss8[:], in1=d2[:],
                                    op=ALU.subtract)
            nc.vector.tensor_reduce(out=lossv[:], in_=loss8[:],
                                    axis=mybir.AxisListType.X, op=ALU.add)
            base, cap = next_slot()
            nc.tensor.matmul(out=mega[0:1, base:base + 1], lhsT=lossv[:],
                             rhs=ones[:], start=True, stop=True)
            nc.scalar.mul(part[0:1, 0:1], mega[0:1, base:base + 1],
                          1.0 / float(B_T))
            nc.sync.dma_start(out=out_ext[:, :], in_=part[:])

    nc.compile()
    return nc


def _get_nc():
    global _NC_CACHE
    if _NC_CACHE is None:
        _NC_CACHE = _build_graph()
    return _NC_CACHE


def _make_in_maps(h, targets, W_head0, W_proj1, W_head1, W_proj2, W_head2):
    FP8NP = ml_dtypes.float8_e4m3
    BF16NP = ml_dtypes.bfloat16
    h = np.ascontiguousarray(np.asarray(h, dtype=np.float32)).reshape(B_T, D)
    t = np.asarray(targets).reshape(-1).astype(np.float32)
    w0 = np.asarray(W_head0, dtype=np.float32)
    w1 = np.asarray(W_head1, dtype=np.float32)
    w2 = np.asarray(W_head2, dtype=np.float32)
    wp1 = np.asarray(W_proj1, dtype=np.float32)
    wp2 = np.asarray(W_proj2, dtype=np.float32)
    w0_8 = np.ascontiguousarray(w0.astype(FP8NP))
    w1_8 = np.ascontiguousarray(w1.astype(FP8NP))
    w2_8 = np.ascontiguousarray(w2.astype(FP8NP))
    wp1_c = np.ascontiguousarray(
        wp1.astype(FP8NP).reshape(8, P, PD1).transpose(1, 0, 2).reshape(
            P, 8 * PD1))
    wp2_c = np.ascontiguousarray(
        wp2.astype(FP8NP).reshape(8, P, PD2).transpose(1, 0, 2).reshape(
            P, 8 * PD2))
    wt0 = np.ascontiguousarray(w0.T.astype(BF16NP))
    wt1 = np.ascontiguousarray(w1.T.astype(BF16NP))
    wt2 = np.ascontiguousarray(w2.T.astype(BF16NP))

    in_maps = []
    for c in range(N_CORES):
        hc = h[c * RPC:(c + 1) * RPC]
        tc_ = t[c * RPC:(c + 1) * RPC]
        ht8 = hc.T.astype(FP8NP).reshape(8, P, RPC).transpose(1, 0, 2)
        in_maps.append({
            "ht": np.ascontiguousarray(ht8.reshape(P, 8 * RPC)),
            "hr": np.ascontiguousarray(hc.astype(BF16NP)),
            "tf": np.ascontiguousarray(tc_.reshape(NRT, P).T),
            "wp1": wp1_c, "wp2": wp2_c,
            "w0": w0_8, "w1": w1_8, "w2": w2_8,
            "wt0": wt0, "wt1": wt1, "wt2": wt2,
        })
    return in_maps


def _finalize(results):
    total = sum(float(results[c]["out"][0, 0]) for c in range(N_CORES))
    return np.float32(total)


def kernel(h, targets, token_to_tier, token_to_idx,
           W_head0, W_proj1, W_head1, W_proj2, W_head2):
    in_maps = _make_in_maps(h, targets, W_head0, W_proj1, W_head1,
                            W_proj2, W_head2)
    nc = _get_nc()
    res = run_bass_kernel_spmd(nc, in_maps, core_ids=list(range(N_CORES)))
    return _finalize(res.results)


# revision 34
# speedup vs baseline: 1.2496x; 1.0152x over previous
"""Adaptive LM head (3-tier chunked softmax cross-entropy) on 8 TRN2 NeuronCores.

Strategy: data-parallel over B_T = 8192 rows (1024 rows/core; weights
replicated, pre-cast to fp8 on the host). The kernel is bound by draining
softmax logits out of PSUM (exp + row-sum of 51.5M elements/core), so that
work is split across both PSUM-capable engines:
  - ScalarE: true exp via one merged activation(Exp, accum_out=...) per
    PSUM round (up to 1536 wide, amortizing the ~400ns instruction+
    accumulator-read overhead).
  - VectorE: Schraudolph fast-exp on tier-pure rounds: one tensor_scalar
    computes int16(round(l*2^7/ln2 + C2_tier)) == the bit pattern of
    bf16(e^l) (C2 calibrated per tier to zero the Z bias), then a bf16
    2x-mode tensor_tensor adds the bits-view into a per-row-tile
    accumulator.
  - TensorE: fp8 DoubleRow matmuls (tiers 0/1) and fp8 matmuls (tier 2)
    into a manually rotated [128,4096] f32 PSUM mega-tile; rounds are
    bank-aligned [1536,1536,1024] so three rounds are in flight.
  - target logit = dot(feature_row, W[:, target]) in bf16: indirect-DMA
    gather of transposed-weight rows + scalar_tensor_tensor multiply-
    reduce, spread through the main stream.
  - per-core partial loss (sum_rows(log Z - target_logit)/8192) is the
    output; the host sums the 8 partials.
"""

import numpy as np
import ml_dtypes

from concourse import bacc, bass, mybir
from concourse.bass import IndirectOffsetOnAxis
from concourse.bass_utils import run_bass_kernel_spmd
from concourse.tile import TileContext

F32 = mybir.dt.float32
BF16 = mybir.dt.bfloat16
I32 = mybir.dt.int32
I16 = mybir.dt.int16
FP8 = mybir.dt.float8e4
DR = mybir.MatmulPerfMode.DoubleRow
ALU = mybir.AluOpType
ACTF = mybir.ActivationFunctionType

P = 128
D = 1024
N_CORES = 8
RPC = 1024          # rows per core
NRT = RPC // P      # row tiles per core = 8
ST = 2048           # vocab super-tile width (DMA granularity)
GW = 512            # psum group width (one f32 PSUM bank)
V0, V1, V2 = 8192, 16384, 25681
PD1, PD2 = 256, 128
B_T = 8192
ZC = 64             # zbigA columns per row-tile
VW = 1536           # vacc columns per row-tile (max V-round width)

# PSUM mega-tile round rotation: bank-aligned offsets/widths.
CYCLE = ((0, 1536), (1536, 1536), (3072, 1024))

# Schraudolph bf16 fast-exp: bits16(e^x) ~= round(x*C1 + C2_t).
# C2 calibrated per tier against the tier's logit sigma so the mean
# multiplicative bias of sum(exp) is ~0 (sigma: t0~0.64, t1~0.21, t2~0.15).
EXP_C1 = float(2.0 ** 7 / np.log(2.0))
EXP_C2 = (16256.0 - 7.25, 16256.0 - 6.5, 16256.0 - 5.5)

# DMA windows: (tier0 st, [tier1 sts], [tier2 sts]) of 2048-col supertiles.
WINDOWS = [
    (0, [0, 1], [0, 1, 2]),
    (1, [2, 3], [3, 4, 5]),
    (2, [4, 5], [6, 7, 8]),
    (3, [6, 7], [9, 10, 11, 12]),
]

G_RTS = ()          # row-tiles whose fast-exp accumulation runs on GpSimd

_NC_CACHE = None


def _ceil_div(a, b):
    return (a + b - 1) // b


def _cost_act(w):
    # merged ACTIVATE + ACTIVATION_READ_ACCUMULATOR + dispatch
    return (w + 222) / 1.2 + 181.0 + 30.0


def _cost_dve(w):
    # pass1 (1x from PSUM) + bf16 TT-add at 2x + dispatch
    return (w + 120) / 0.96 + (w / 2 + 58) / 0.96 + 120.0


def _build_graph():
    nc = bacc.Bacc("TRN2", target_bir_lowering=False, debug=False,
                   num_devices=N_CORES)

    ht_ext = nc.declare_dram_parameter("ht", [P, 8 * RPC], FP8, isOutput=False)
    hr_ext = nc.declare_dram_parameter("hr", [RPC, D], BF16, isOutput=False)
    tf_ext = nc.declare_dram_parameter("tf", [P, NRT], F32, isOutput=False)
    wp1_ext = nc.declare_dram_parameter("wp1", [P, 8 * PD1], FP8, isOutput=False)
    wp2_ext = nc.declare_dram_parameter("wp2", [P, 8 * PD2], FP8, isOutput=False)
    w0_ext = nc.declare_dram_parameter("w0", [D, V0], FP8, isOutput=False)
    w1_ext = nc.declare_dram_parameter("w1", [PD1, V1], FP8, isOutput=False)
    w2_ext = nc.declare_dram_parameter("w2", [PD2, V2], FP8, isOutput=False)
    wt0_ext = nc.declare_dram_parameter("wt0", [V0, D], BF16, isOutput=False)
    wt1_ext = nc.declare_dram_parameter("wt1", [V1, PD1], BF16, isOutput=False)
    wt2_ext = nc.declare_dram_parameter("wt2", [V2, PD2], BF16, isOutput=False)
    out_ext = nc.declare_dram_parameter("out", [1, 1], F32, isOutput=True)

    with TileContext(nc) as tc:
        with (
            tc.tile_pool(name="res", bufs=1) as res,
            tc.tile_pool(name="w0pool", bufs=2) as w0pool,
            tc.tile_pool(name="w1pool", bufs=4) as w1pool,
            tc.tile_pool(name="w2pool", bufs=6) as w2pool,
            tc.tile_pool(name="hrpool", bufs=2) as hrpool,
            tc.tile_pool(name="expool", bufs=3) as expool,
            tc.tile_pool(name="e16pool", bufs=6) as e16pool,
            tc.tile_pool(name="gpool", bufs=2) as gpool,
            tc.tile_pool(name="prodpool", bufs=2) as prodpool,
            tc.tile_pool(name="psum", bufs=1, space="PSUM") as psum,
        ):
            # ---------------- resident tiles ----------------
            ht8_sb = res.tile([P, 8 * RPC], FP8, tag="ht8")
            wp1_8 = res.tile([P, 8 * PD1], FP8, tag="wp18")
            wp2_8 = res.tile([P, 8 * PD2], FP8, tag="wp28")
            hp1T_sb = res.tile([P, 2 * RPC], FP8, tag="hp1T")
            hp2T_sb = res.tile([P, 1 * RPC], FP8, tag="hp2T")
            hp1r_sb = res.tile([P, NRT * PD1], BF16, tag="hp1r")
            hp2r_sb = res.tile([P, NRT * PD2], BF16, tag="hp2r")
            tf_sb = res.tile([P, NRT], F32, tag="tf")
            ge1 = res.tile([P, NRT], F32, tag="ge1")
            ge2 = res.tile([P, NRT], F32, tag="ge2")
            idxf = [res.tile([P, NRT], F32, tag=f"idxf{t}", name=f"idxf{t}")
                    for t in range(3)]
            idxi = [res.tile([P, NRT], I32, tag=f"idxi{t}", name=f"idxi{t}")
                    for t in range(3)]
            tl = [res.tile([P, NRT], F32, tag=f"tl{t}", name=f"tl{t}")
                  for t in range(3)]
            zbigA = res.tile([P, NRT * ZC], F32, tag="zbigA")
            vacc = res.tile([P, NRT * VW], BF16, tag="vacc")
            zredA = res.tile([P, NRT], F32, tag="zredA")
            zredV = res.tile([P, NRT], F32, tag="zredV")
            zred = res.tile([P, NRT], F32, tag="zred")
            logz = res.tile([P, NRT], F32, tag="logz")
            d1 = res.tile([P, NRT], F32, tag="d1")
            d2 = res.tile([P, NRT], F32, tag="d2")
            loss8 = res.tile([P, NRT], F32, tag="loss8")
            lossv = res.tile([P, 1], F32, tag="lossv")
            ones = res.tile([P, 1], F32, tag="ones")
            part = res.tile([1, 1], F32, tag="part")

            mega = psum.tile([P, 4096], F32, tag="mega")

            # fp8 staging: host pre-chunked layouts, split across the
            # HWDGE (sync) and SWDGE (gpsimd) queues so ht8 (1MB, gates
            # the hp2T projection) lands as early as possible
            nc.sync.dma_start(out=ht8_sb[:, 0:4 * RPC],
                              in_=ht_ext[:, 0:4 * RPC])
            nc.gpsimd.dma_start(out=ht8_sb[:, 4 * RPC:8 * RPC],
                                in_=ht_ext[:, 4 * RPC:8 * RPC])
            nc.sync.dma_start(out=wp2_8[:], in_=wp2_ext[:, :])
            nc.sync.dma_start(out=wp1_8[:], in_=wp1_ext[:, :])
            nc.sync.dma_start(out=tf_sb[:], in_=tf_ext[:, :])

            nc.scalar.memzero(zbigA[:])
            nc.vector.memset(ones[:], 1.0)
            warm = res.tile([1, 1], F32, tag="warm")
            nc.scalar.activation(warm[0:1, 0:1], ones[0:1, 0:1], ACTF.Exp)
            # PE warmup: ~40 tiny matmuls on a garbage tile unthrottle the
            # HAM clock gate (~3.4us of activity) before the real stream
            junk = res.tile([P, 4 * P], FP8, tag="junk")
            nc.vector.memset(junk[:], 0.0)
            for wi_ in range(40):
                nc.tensor.matmul(
                    out=mega[:, 3072 + (wi_ % 2) * GW:
                             3072 + (wi_ % 2) * GW + P],
                    lhsT=junk[:, 0:P], rhs=junk[:, (wi_ % 3) * P:
                                                (wi_ % 3) * P + P],
                    start=True, stop=True)

            # ---------------- masks and in-tier indices ----------------
            nc.vector.tensor_scalar(out=ge1[:], in0=tf_sb[:], scalar1=float(V0),
                                    scalar2=None, op0=ALU.is_ge)
            nc.vector.tensor_scalar(out=ge2[:], in0=tf_sb[:],
                                    scalar1=float(V0 + V1), scalar2=None,
                                    op0=ALU.is_ge)
            nc.vector.tensor_scalar(out=idxf[0][:], in0=tf_sb[:],
                                    scalar1=float(V0 - 1), scalar2=None,
                                    op0=ALU.min)
            nc.vector.tensor_scalar(out=idxf[1][:], in0=tf_sb[:],
                                    scalar1=-float(V0), scalar2=0.0,
                                    op0=ALU.add, op1=ALU.max)
            nc.vector.tensor_scalar(out=idxf[1][:], in0=idxf[1][:],
                                    scalar1=float(V1 - 1), scalar2=None,
                                    op0=ALU.min)
            nc.vector.tensor_scalar(out=idxf[2][:], in0=tf_sb[:],
                                    scalar1=-float(V0 + V1), scalar2=0.0,
                                    op0=ALU.add, op1=ALU.max)
            nc.vector.tensor_scalar(out=idxf[2][:], in0=idxf[2][:],
                                    scalar1=float(V2 - 1), scalar2=None,
                                    op0=ALU.min)
            for t in range(3):
                nc.vector.tensor_copy(out=idxi[t][:], in_=idxf[t][:])

            ht8v = ht8_sb[:].rearrange("p (k r) -> p k r", k=8)
            wp18v = wp1_8[:].rearrange("p (k c) -> p k c", k=8)
            wp28v = wp2_8[:].rearrange("p (k c) -> p k c", k=8)
            hp1Tv = hp1T_sb[:].rearrange("p (k r) -> p k r", k=2)

            # round-slot rotation over the mega tile
            slot_i = [0]

            def next_slot():
                off, w = CYCLE[slot_i[0] % 3]
                slot_i[0] += 1
                return off, w

            # greedy engine-balance state (ns)
            eng_t = {"A": 0.0, "V": 12000.0}
            zcols = [0] * NRT

            # ---------------- hp2T projection (runway prerequisite) -------
            base, cap = next_slot()
            for g in range(2):
                for pr in range(4):
                    nc.tensor.matmul(
                        out=mega[:, base + g * GW: base + (g + 1) * GW],
                        lhsT=wp28v[:, 2 * pr: 2 * pr + 2, 0:P],
                        rhs=ht8v[:, 2 * pr: 2 * pr + 2, g * GW:(g + 1) * GW],
                        start=(pr == 0), stop=(pr == 3), perf_mode=DR)
            nc.vector.tensor_copy(out=hp2T_sb[:],
                                  in_=mega[:, base: base + RPC])
            eng_t["V"] += (RPC + 120) / 0.96 + 60

            # ---------------- main stream ----------------
            # tier -> (V, Kchunks, w_ext, wpool, doublerow)
            tiers = {
                0: (V0, 8, w0_ext, w0pool, True),
                1: (V1, 2, w1_ext, w1pool, True),
                2: (V2, 1, w2_ext, w2pool, False),
            }
            gather_src = [wt0_ext, wt1_ext, wt2_ext]
            gdim = [D, PD1, PD2]
            gmax = [V0 - 1, V1 - 1, V2 - 1]
            st_wtile = {}

            def ensure_st(tier, st):
                if (tier, st) in st_wtile:
                    return
                V, K, w_ext, wpool, dr = tiers[tier]
                w = min(ST, V - st * ST)
                wtile = wpool.tile([P, K * ST], FP8,
                                   tag=f"w{tier}", name=f"w{tier}")
                for k in range(K):
                    nc.gpsimd.dma_start(
                        out=wtile[:, k * ST: k * ST + w],
                        in_=w_ext[k * P:(k + 1) * P, st * ST: st * ST + w])
                st_wtile[(tier, st)] = wtile

            def st_groups(tier, st):
                V = tiers[tier][0]
                w = min(ST, V - st * ST)
                return [(tier, st, g, min(GW, w - g * GW))
                        for g in range(_ceil_div(w, GW))]

            def emit_round(groups, rt, useV):
                base, cap = next_slot()
                off = 0
                for (tier, st, g, gw) in groups:
                    V, K, w_ext, wpool, dr = tiers[tier]
                    wtile = st_wtile[(tier, st)]
                    dst = mega[:, base + off: base + off + gw]
                    if dr:
                        wv = wtile[:].rearrange("p (k c) -> p k c", k=K)
                        lv = ht8v if tier == 0 else hp1Tv
                        for pr in range(K // 2):
                            nc.tensor.matmul(
                                out=dst,
                                lhsT=lv[:, 2 * pr: 2 * pr + 2,
                                        rt * P: rt * P + P],
                                rhs=wv[:, 2 * pr: 2 * pr + 2,
                                       g * GW: g * GW + gw],
                                start=(pr == 0), stop=(pr == K // 2 - 1),
                                perf_mode=DR)
                    else:
                        nc.tensor.matmul(
                            out=dst,
                            lhsT=hp2T_sb[:, rt * P: rt * P + P],
                            rhs=wtile[:, g * GW: g * GW + gw],
                            start=True, stop=True)
                    off += gw
                src = mega[:, base: base + off]
                if useV:
                    tier = groups[0][0]
                    e16 = e16pool.tile([P, 1536], I16, tag="e16")
                    nc.vector.tensor_scalar(
                        out=e16[:, :off], in0=src,
                        scalar1=EXP_C1, scalar2=EXP_C2[tier],
                        op0=ALU.mult, op1=ALU.add)
                    va = vacc[:, rt * VW: rt * VW + off]
                    if rt in G_RTS:
                        nc.gpsimd.tensor_tensor(
                            out=va, in0=va, in1=e16[:, :off].bitcast(BF16),
                            op=ALU.add)
                        eng_t["V"] += (off + 120) / 0.96 + 60.0
                    else:
                        nc.vector.tensor_tensor(
                            out=va, in0=va, in1=e16[:, :off].bitcast(BF16),
                            op=ALU.add)
                        eng_t["V"] += _cost_dve(off)
                else:
                    zcol = rt * ZC + zcols[rt]
                    zcols[rt] += 1
                    ex = expool.tile([P, 1536], BF16, tag="ex")
                    nc.scalar.activation(
                        ex[:, :off], src, ACTF.Exp,
                        accum_out=zbigA[:, zcol: zcol + 1])
                    eng_t["A"] += _cost_act(off)

            last_eng = ["A"]

            def plan_emit(tier_lists, rt):
                # per-tier queues; rounds draw via largest-remaining-fraction
                qs = [list(l) for l in tier_lists if l]
                tot = [len(q) for q in qs]
                while any(qs):
                    cap = CYCLE[slot_i[0] % 3][1]
                    nfit = cap // GW
                    # tier-pure candidate for a V round: tier with the most
                    # remaining groups
                    vi = max(range(len(qs)), key=lambda j: len(qs[j]))
                    vgroups = qs[vi][:nfit]
                    # A-round candidate: Bresenham across tiers
                    apick = []
                    idx = [0] * len(qs)
                    for _ in range(nfit):
                        best, bj = -1.0, -1
                        for j, q in enumerate(qs):
                            rema = len(q) - idx[j]
                            if rema > 0 and rema / tot[j] > best:
                                best, bj = rema / tot[j], j
                        if bj < 0:
                            break
                        apick.append((bj, idx[bj]))
                        idx[bj] += 1
                    agroups = [qs[j][k] for (j, k) in apick]
                    wV = sum(g[3] for g in vgroups)
                    wA = sum(g[3] for g in agroups)
                    # alternation bias against same-engine streaks
                    bias = 250.0 if last_eng[0] == "A" else -250.0
                    useV = bool(vgroups) and (
                        eng_t["V"] + _cost_dve(wV) - bias <
                        eng_t["A"] + _cost_act(wA))
                    if useV:
                        qs[vi] = qs[vi][nfit:]
                        emit_round(vgroups, rt, True)
                        last_eng[0] = "V"
                    else:
                        for (j, k) in sorted(apick, reverse=True):
                            qs[j].pop(k)
                        emit_round(agroups, rt, False)
                        last_eng[0] = "A"

            def emit_rt_final(rt):
                # row-tile Z reduction, emitted as soon as rt's stream ends
                nc.vector.tensor_reduce(
                    out=zredA[:, rt:rt + 1],
                    in_=zbigA[:, rt * ZC:(rt + 1) * ZC],
                    axis=mybir.AxisListType.X, op=ALU.add)
                eng_t["V"] += (ZC + 58) / 0.96 + 60
                cA = (VW + 224) / 1.2 + 181
                cV = (VW + 58) / 0.96
                if eng_t["A"] + cA < eng_t["V"] + cV:
                    ex = expool.tile([P, 1536], BF16, tag="ex")
                    nc.scalar.activation(
                        ex[:, :VW], vacc[:, rt * VW:(rt + 1) * VW],
                        ACTF.Identity, accum_out=zredV[:, rt:rt + 1])
                    eng_t["A"] += cA
                else:
                    nc.vector.tensor_reduce(
                        out=zredV[:, rt:rt + 1],
                        in_=vacc[:, rt * VW:(rt + 1) * VW],
                        axis=mybir.AxisListType.X, op=ALU.add)
                    eng_t["V"] += cV

            def emit_rows_proj(rt, t):
                # DR rows-orientation projection feeding the target dot
                pd = PD1 if t == 1 else PD2
                wv = wp18v if t == 1 else wp28v
                dstt = hp1r_sb if t == 1 else hp2r_sb
                base, cap = next_slot()
                for pr in range(4):
                    nc.tensor.matmul(
                        out=mega[:, base: base + pd],
                        lhsT=ht8v[:, 2 * pr: 2 * pr + 2,
                                  rt * P: rt * P + P],
                        rhs=wv[:, 2 * pr: 2 * pr + 2, 0:pd],
                        start=(pr == 0), stop=(pr == 3), perf_mode=DR)
                nc.vector.tensor_copy(
                    out=dstt[:, rt * pd:(rt + 1) * pd],
                    in_=mega[:, base: base + pd])
                eng_t["V"] += (pd + 120) / 0.96 + 60

            def emit_gather_dot(i):
                rt, t = divmod(i, 3)
                if t == 0:
                    hr_t = hrpool.tile([P, D], BF16, tag="hrt", name="hrt")
                    nc.sync.dma_start(out=hr_t[:],
                                      in_=hr_ext[rt * P:(rt + 1) * P, :])
                    feat_ap = hr_t[:]
                elif t == 1:
                    emit_rows_proj(rt, 1)
                    feat_ap = hp1r_sb[:, rt * PD1:(rt + 1) * PD1]
                else:
                    emit_rows_proj(rt, 2)
                    feat_ap = hp2r_sb[:, rt * PD2:(rt + 1) * PD2]
                g = gpool.tile([P, gdim[t]], BF16, tag=f"g{t}", name=f"g{t}")
                nc.gpsimd.indirect_dma_start(
                    out=g[:], out_offset=None,
                    in_=gather_src[t][:, :],
                    in_offset=IndirectOffsetOnAxis(
                        ap=idxi[t][:, rt:rt + 1], axis=0),
                    bounds_check=gmax[t], oob_is_err=False)
                prod = prodpool.tile([P, D], BF16, tag="prod")
                nc.vector.scalar_tensor_tensor(
                    out=prod[:, :gdim[t]],
                    in0=feat_ap, scalar=1.0, in1=g[:],
                    op0=ALU.mult, op1=ALU.mult,
                    accum_out=tl[t][:, rt:rt + 1])
                eng_t["V"] += (gdim[t] / 2 + 58) / 0.96 + 60

            def interleave(lists):
                # Bresenham-style proportional merge of per-tier group lists
                out = []
                idx = [0] * len(lists)
                tot = [len(l) for l in lists]
                n = sum(tot)
                for _ in range(n):
                    best, bi = -1.0, 0
                    for j, l in enumerate(lists):
                        if idx[j] < tot[j]:
                            frac = (tot[j] - idx[j]) / tot[j]
                            if frac > best:
                                best, bi = frac, j
                    out.append(lists[bi][idx[bi]])
                    idx[bi] += 1
                return out

            gi = 0
            for wi, (a_st, b_sts, c_sts) in enumerate(WINDOWS):
                for st in c_sts:
                    ensure_st(2, st)
                ensure_st(0, a_st)
                for st in b_sts:
                    ensure_st(1, st)
                As = st_groups(0, a_st)
                Bs = [g for st in b_sts for g in st_groups(1, st)]
                Cs = [g for st in c_sts for g in st_groups(2, st)]
                if wi == 0:
                    # runway: tier2 rounds only while w0/w1 land; vacc
                    # slices are zeroed here (V is otherwise idle early)
                    for rt in range(NRT):
                        # zero vacc on ScalarE (idle during the runway ramp)
                        nc.scalar.memzero(vacc[:, rt * VW:(rt + 1) * VW])
                        plan_emit([Cs[0:8]], rt)
                    # hp1T projection: needed by the first B rounds
                    for m in range(2):
                        base, cap = next_slot()
                        for g in range(2):
                            for pr in range(4):
                                nc.tensor.matmul(
                                    out=mega[:, base + g * GW:
                                             base + (g + 1) * GW],
                                    lhsT=wp18v[:, 2 * pr: 2 * pr + 2,
                                               m * P:(m + 1) * P],
                                    rhs=ht8v[:, 2 * pr: 2 * pr + 2,
                                             g * GW:(g + 1) * GW],
                                    start=(pr == 0), stop=(pr == 3),
                                    perf_mode=DR)
                        nc.vector.tensor_copy(
                            out=hp1T_sb[:, m * RPC:(m + 1) * RPC],
                            in_=mega[:, base: base + RPC])
                        eng_t["V"] += (RPC + 120) / 0.96 + 60
                    for rt in range(NRT):
                        plan_emit([Cs[8:], As, Bs], rt)
                    continue
                for rt in range(NRT):
                    plan_emit([As, Bs, Cs], rt)
                    if gi < 3 * NRT:
                        emit_gather_dot(gi)
                        gi += 1
                    if wi == 3:
                        emit_rt_final(rt)
            while gi < 3 * NRT:
                emit_gather_dot(gi)
                gi += 1

            # ---------------- final reduction ----------------
            # zred = zredA + zredV + d1 (d1 holds the ScalarE-reduced
            # second vacc half where that path was taken)
            nc.vector.tensor_tensor(out=zred[:], in0=zredA[:], in1=zredV[:],
                                    op=ALU.add)
            nc.scalar.activation(logz[:], zred[:], ACTF.Ln)
            # loss8 = logz - (tl0 + ge1*(tl1-tl0) + ge2*(tl2-tl1))
            nc.vector.tensor_tensor(out=d1[:], in0=tl[1][:], in1=tl[0][:],
                                    op=ALU.subtract)
            nc.vector.tensor_tensor(out=d2[:], in0=tl[2][:], in1=tl[1][:],
                                    op=ALU.subtract)
            nc.vector.tensor_tensor(out=d1[:], in0=d1[:], in1=ge1[:],
                                    op=ALU.mult)
            nc.vector.tensor_tensor(out=d2[:], in0=d2[:], in1=ge2[:],
                                    op=ALU.mult)
            nc.vector.tensor_tensor(out=loss8[:], in0=logz[:], in1=tl[0][:],
                                    op=ALU.subtract)
            nc.vector.tensor_tensor(out=loss8[:], in0=loss8[:], in1=d1[:],
                                    op=ALU.subtract)
            nc.vector.tensor_tensor(out=loss8[:], in0=loss8[:], in1=d2[:],
                                    op=ALU.subtract)
            nc.vector.tensor_reduce(out=lossv[:], in_=loss8[:],
                                    axis=mybir.AxisListType.X, op=ALU.add)
            base, cap = next_slot()
            nc.tensor.matmul(out=mega[0:1, base:base + 1], lhsT=lossv[:],
                             rhs=ones[:], start=True, stop=True)
            nc.scalar.mul(part[0:1, 0:1], mega[0:1, base:base + 1],
                          1.0 / float(B_T))
            nc.sync.dma_start(out=out_ext[:, :], in_=part[:])

    nc.compile()
    return nc


def _get_nc():
    global _NC_CACHE
    if _NC_CACHE is None:
        _NC_CACHE = _build_graph()
    return _NC_CACHE


def _make_in_maps(h, targets, W_head0, W_proj1, W_head1, W_proj2, W_head2):
    FP8NP = ml_dtypes.float8_e4m3
    BF16NP = ml_dtypes.bfloat16
    h = np.ascontiguousarray(np.asarray(h, dtype=np.float32)).reshape(B_T, D)
    t = np.asarray(targets).reshape(-1).astype(np.float32)
    w0 = np.asarray(W_head0, dtype=np.float32)
    w1 = np.asarray(W_head1, dtype=np.float32)
    w2 = np.asarray(W_head2, dtype=np.float32)
    wp1 = np.asarray(W_proj1, dtype=np.float32)
    wp2 = np.asarray(W_proj2, dtype=np.float32)
    w0_8 = np.ascontiguousarray(w0.astype(FP8NP))
    w1_8 = np.ascontiguousarray(w1.astype(FP8NP))
    w2_8 = np.ascontiguousarray(w2.astype(FP8NP))
    wp1_c = np.ascontiguousarray(
        wp1.astype(FP8NP).reshape(8, P, PD1).transpose(1, 0, 2).reshape(
            P, 8 * PD1))
    wp2_c = np.ascontiguousarray(
        wp2.astype(FP8NP).reshape(8, P, PD2).transpose(1, 0, 2).reshape(
            P, 8 * PD2))
    wt0 = np.ascontiguousarray(w0.T.astype(BF16NP))
    wt1 = np.ascontiguousarray(w1.T.astype(BF16NP))
    wt2 = np.ascontiguousarray(w2.T.astype(BF16NP))

    in_maps = []
    for c in range(N_CORES):
        hc = h[c * RPC:(c + 1) * RPC]
        tc_ = t[c * RPC:(c + 1) * RPC]
        ht8 = hc.T.astype(FP8NP).reshape(8, P, RPC).transpose(1, 0, 2)
        in_maps.append({
            "ht": np.ascontiguousarray(ht8.reshape(P, 8 * RPC)),
            "hr": np.ascontiguousarray(hc.astype(BF16NP)),
            "tf": np.ascontiguousarray(tc_.reshape(NRT, P).T),
            "wp1": wp1_c, "wp2": wp2_c,
            "w0": w0_8, "w1": w1_8, "w2": w2_8,
            "wt0": wt0, "wt1": wt1, "wt2": wt2,
        })
    return in_maps


def _finalize(results):
    total = sum(float(results[c]["out"][0, 0]) for c in range(N_CORES))
    return np.float32(total)


def kernel(h, targets, token_to_tier, token_to_idx,
           W_head0, W_proj1, W_head1, W_proj2, W_head2):
    in_maps = _make_in_maps(h, targets, W_head0, W_proj1, W_head1,
                            W_proj2, W_head2)
    nc = _get_nc()
    res = run_bass_kernel_spmd(nc, in_maps, core_ids=list(range(N_CORES)))
    return _finalize(res.results)


# revision 35
# speedup vs baseline: 1.2753x; 1.0205x over previous
"""Adaptive LM head (3-tier chunked softmax cross-entropy) on 8 TRN2 NeuronCores.

Strategy: data-parallel over B_T = 8192 rows (1024 rows/core; weights
replicated, pre-cast to fp8 on the host). The kernel is bound by draining
softmax logits out of PSUM (exp + row-sum of 51.5M elements/core), so that
work is split across both PSUM-capable engines:
  - ScalarE: true exp via one merged activation(Exp, accum_out=...) per
    PSUM round (up to 1536 wide, amortizing the ~400ns instruction+
    accumulator-read overhead).
  - VectorE: Schraudolph fast-exp on tier-pure rounds: one tensor_scalar
    computes int16(round(l*2^7/ln2 + C2_tier)) == the bit pattern of
    bf16(e^l) (C2 calibrated per tier to zero the Z bias), then a bf16
    2x-mode tensor_tensor adds the bits-view into a per-row-tile
    accumulator.
  - TensorE: fp8 DoubleRow matmuls (tiers 0/1) and fp8 matmuls (tier 2)
    into a manually rotated [128,4096] f32 PSUM mega-tile; rounds are
    bank-aligned [1536,1536,1024] so three rounds are in flight.
  - target logit = dot(feature_row, W[:, target]) in bf16: indirect-DMA
    gather of transposed-weight rows + scalar_tensor_tensor multiply-
    reduce, spread through the main stream.
  - per-core partial loss (sum_rows(log Z - target_logit)/8192) is the
    output; the host sums the 8 partials.
"""

import numpy as np
import ml_dtypes

from concourse import bacc, bass, mybir
from concourse.bass import IndirectOffsetOnAxis
from concourse.bass_utils import run_bass_kernel_spmd
from concourse.tile import TileContext

F32 = mybir.dt.float32
BF16 = mybir.dt.bfloat16
I32 = mybir.dt.int32
I16 = mybir.dt.int16
FP8 = mybir.dt.float8e4
DR = mybir.MatmulPerfMode.DoubleRow
ALU = mybir.AluOpType
ACTF = mybir.ActivationFunctionType

P = 128
D = 1024
N_CORES = 8
RPC = 1024          # rows per core
NRT = RPC // P      # row tiles per core = 8
ST = 2048           # vocab super-tile width (DMA granularity)
GW = 512            # psum group width (one f32 PSUM bank)
V0, V1, V2 = 8192, 16384, 25681
PD1, PD2 = 256, 128
B_T = 8192
ZC = 64             # zbigA columns per row-tile
VW = 1536           # vacc columns per row-tile (max V-round width)

# PSUM mega-tile round rotation: bank-aligned offsets/widths.
CYCLE = ((0, 1536), (1536, 1536), (3072, 1024))

# Schraudolph bf16 fast-exp: bits16(e^x) ~= round(x*C1 + C2_t).
# C2 calibrated per tier against the tier's logit sigma so the mean
# multiplicative bias of sum(exp) is ~0 (sigma: t0~0.64, t1~0.21, t2~0.15).
EXP_C1 = float(2.0 ** 7 / np.log(2.0))
EXP_C2 = (16256.0 - 7.25, 16256.0 - 6.5, 16256.0 - 5.5)

# DMA windows: (tier0 st, [tier1 sts], [tier2 sts]) of 2048-col supertiles.
WINDOWS = [
    (0, [0, 1], [0, 1, 2]),
    (1, [2, 3], [3, 4, 5]),
    (2, [4, 5], [6, 7, 8]),
    (3, [6, 7], [9, 10, 11, 12]),
]

G_RTS = ()          # row-tiles whose fast-exp accumulation runs on GpSimd

_NC_CACHE = None


def _ceil_div(a, b):
    return (a + b - 1) // b


def _cost_act(w):
    # merged ACTIVATE + ACTIVATION_READ_ACCUMULATOR + dispatch
    return (w + 222) / 1.2 + 181.0 + 30.0


def _cost_dve(w):
    # pass1 (1x from PSUM) + bf16 TT-add at 2x + dispatch
    return (w + 120) / 0.96 + (w / 2 + 58) / 0.96 + 120.0


def _build_graph():
    nc = bacc.Bacc("TRN2", target_bir_lowering=False, debug=False,
                   num_devices=N_CORES)

    ht_ext = nc.declare_dram_parameter("ht", [P, 8 * RPC], FP8, isOutput=False)
    hr_ext = nc.declare_dram_parameter("hr", [RPC, D], BF16, isOutput=False)
    tf_ext = nc.declare_dram_parameter("tf", [P, NRT], F32, isOutput=False)
    wp1_ext = nc.declare_dram_parameter("wp1", [P, 8 * PD1], FP8, isOutput=False)
    wp2_ext = nc.declare_dram_parameter("wp2", [P, 8 * PD2], FP8, isOutput=False)
    w0_ext = nc.declare_dram_parameter("w0", [D, V0], FP8, isOutput=False)
    w1_ext = nc.declare_dram_parameter("w1", [PD1, V1], FP8, isOutput=False)
    w2_ext = nc.declare_dram_parameter("w2", [PD2, V2], FP8, isOutput=False)
    wt0_ext = nc.declare_dram_parameter("wt0", [V0, D], BF16, isOutput=False)
    wt1_ext = nc.declare_dram_parameter("wt1", [V1, PD1], BF16, isOutput=False)
    wt2_ext = nc.declare_dram_parameter("wt2", [V2, PD2], BF16, isOutput=False)
    out_ext = nc.declare_dram_parameter("out", [1, 1], F32, isOutput=True)

    with TileContext(nc) as tc:
        with (
            tc.tile_pool(name="res", bufs=1) as res,
            tc.tile_pool(name="w0pool", bufs=2) as w0pool,
            tc.tile_pool(name="w1pool", bufs=4) as w1pool,
            tc.tile_pool(name="w2pool", bufs=6) as w2pool,
            tc.tile_pool(name="hrpool", bufs=2) as hrpool,
            tc.tile_pool(name="expool", bufs=3) as expool,
            tc.tile_pool(name="e16pool", bufs=6) as e16pool,
            tc.tile_pool(name="gpool", bufs=2) as gpool,
            tc.tile_pool(name="prodpool", bufs=2) as prodpool,
            tc.tile_pool(name="psum", bufs=1, space="PSUM") as psum,
        ):
            # ---------------- resident tiles ----------------
            ht8_sb = res.tile([P, 8 * RPC], FP8, tag="ht8")
            wp1_8 = res.tile([P, 8 * PD1], FP8, tag="wp18")
            wp2_8 = res.tile([P, 8 * PD2], FP8, tag="wp28")
            hp1T_sb = res.tile([P, 2 * RPC], FP8, tag="hp1T")
            hp2T_sb = res.tile([P, 1 * RPC], FP8, tag="hp2T")
            hp1r_sb = res.tile([P, NRT * PD1], BF16, tag="hp1r")
            hp2r_sb = res.tile([P, NRT * PD2], BF16, tag="hp2r")
            tf_sb = res.tile([P, NRT], F32, tag="tf")
            ge1 = res.tile([P, NRT], F32, tag="ge1")
            ge2 = res.tile([P, NRT], F32, tag="ge2")
            idxf = [res.tile([P, NRT], F32, tag=f"idxf{t}", name=f"idxf{t}")
                    for t in range(3)]
            idxi = [res.tile([P, NRT], I32, tag=f"idxi{t}", name=f"idxi{t}")
                    for t in range(3)]
            tl = [res.tile([P, NRT], F32, tag=f"tl{t}", name=f"tl{t}")
                  for t in range(3)]
            zbigA = res.tile([P, NRT * ZC], F32, tag="zbigA")
            vacc = res.tile([P, NRT * VW], BF16, tag="vacc")
            zredA = res.tile([P, NRT], F32, tag="zredA")
            zredV = res.tile([P, NRT], F32, tag="zredV")
            zred = res.tile([P, NRT], F32, tag="zred")
            logz = res.tile([P, NRT], F32, tag="logz")
            d1 = res.tile([P, NRT], F32, tag="d1")
            d2 = res.tile([P, NRT], F32, tag="d2")
            loss8 = res.tile([P, NRT], F32, tag="loss8")
            lossv = res.tile([P, 1], F32, tag="lossv")
            ones = res.tile([P, 1], F32, tag="ones")
            part = res.tile([1, 1], F32, tag="part")

            mega = psum.tile([P, 4096], F32, tag="mega")

            # fp8 staging: host pre-chunked layouts, split across the
            # HWDGE (sync) and SWDGE (gpsimd) queues so ht8 (1MB, gates
            # the hp2T projection) lands as early as possible
            nc.sync.dma_start(out=ht8_sb[:, 0:4 * RPC],
                              in_=ht_ext[:, 0:4 * RPC])
            nc.gpsimd.dma_start(out=ht8_sb[:, 4 * RPC:8 * RPC],
                                in_=ht_ext[:, 4 * RPC:8 * RPC])
            nc.sync.dma_start(out=wp2_8[:], in_=wp2_ext[:, :])
            nc.sync.dma_start(out=wp1_8[:], in_=wp1_ext[:, :])
            nc.sync.dma_start(out=tf_sb[:], in_=tf_ext[:, :])

            nc.scalar.memzero(zbigA[:])
            nc.vector.memset(ones[:], 1.0)
            warm = res.tile([1, 1], F32, tag="warm")
            nc.scalar.activation(warm[0:1, 0:1], ones[0:1, 0:1], ACTF.Exp)
            # PE warmup: ~40 tiny matmuls on a garbage tile unthrottle the
            # HAM clock gate (~3.4us of activity) before the real stream
            junk = res.tile([P, 4 * P], FP8, tag="junk")
            nc.vector.memset(junk[:], 0.0)
            for wi_ in range(40):
                nc.tensor.matmul(
                    out=mega[:, 3072 + (wi_ % 2) * GW:
                             3072 + (wi_ % 2) * GW + P],
                    lhsT=junk[:, 0:P], rhs=junk[:, (wi_ % 3) * P:
                                                (wi_ % 3) * P + P],
                    start=True, stop=True)

            # ---------------- masks and in-tier indices ----------------
            nc.vector.tensor_scalar(out=ge1[:], in0=tf_sb[:], scalar1=float(V0),
                                    scalar2=None, op0=ALU.is_ge)
            nc.vector.tensor_scalar(out=ge2[:], in0=tf_sb[:],
                                    scalar1=float(V0 + V1), scalar2=None,
                                    op0=ALU.is_ge)
            nc.vector.tensor_scalar(out=idxf[0][:], in0=tf_sb[:],
                                    scalar1=float(V0 - 1), scalar2=None,
                                    op0=ALU.min)
            nc.vector.tensor_scalar(out=idxf[1][:], in0=tf_sb[:],
                                    scalar1=-float(V0), scalar2=0.0,
                                    op0=ALU.add, op1=ALU.max)
            nc.vector.tensor_scalar(out=idxf[1][:], in0=idxf[1][:],
                                    scalar1=float(V1 - 1), scalar2=None,
                                    op0=ALU.min)
            nc.vector.tensor_scalar(out=idxf[2][:], in0=tf_sb[:],
                                    scalar1=-float(V0 + V1), scalar2=0.0,
                                    op0=ALU.add, op1=ALU.max)
            nc.vector.tensor_scalar(out=idxf[2][:], in0=idxf[2][:],
                                    scalar1=float(V2 - 1), scalar2=None,
                                    op0=ALU.min)
            for t in range(3):
                nc.vector.tensor_copy(out=idxi[t][:], in_=idxf[t][:])

            ht8v = ht8_sb[:].rearrange("p (k r) -> p k r", k=8)
            wp18v = wp1_8[:].rearrange("p (k c) -> p k c", k=8)
            wp28v = wp2_8[:].rearrange("p (k c) -> p k c", k=8)
            hp1Tv = hp1T_sb[:].rearrange("p (k r) -> p k r", k=2)

            # round-slot rotation over the mega tile
            slot_i = [0]

            def next_slot():
                off, w = CYCLE[slot_i[0] % 3]
                slot_i[0] += 1
                return off, w

            # greedy engine-balance state (ns)
            eng_t = {"A": 0.0, "V": 12000.0}
            zcols = [0] * NRT

            # ---------------- hp2T projection (runway prerequisite) -------
            base, cap = next_slot()
            for g in range(2):
                for pr in range(4):
                    nc.tensor.matmul(
                        out=mega[:, base + g * GW: base + (g + 1) * GW],
                        lhsT=wp28v[:, 2 * pr: 2 * pr + 2, 0:P],
                        rhs=ht8v[:, 2 * pr: 2 * pr + 2, g * GW:(g + 1) * GW],
                        start=(pr == 0), stop=(pr == 3), perf_mode=DR)
            nc.vector.tensor_copy(out=hp2T_sb[:],
                                  in_=mega[:, base: base + RPC])
            eng_t["V"] += (RPC + 120) / 0.96 + 60

            # ---------------- main stream ----------------
            # tier -> (V, Kchunks, w_ext, wpool, doublerow)
            tiers = {
                0: (V0, 8, w0_ext, w0pool, True),
                1: (V1, 2, w1_ext, w1pool, True),
                2: (V2, 1, w2_ext, w2pool, False),
            }
            gather_src = [wt0_ext, wt1_ext, wt2_ext]
            gdim = [D, PD1, PD2]
            gmax = [V0 - 1, V1 - 1, V2 - 1]
            st_wtile = {}

            def ensure_st(tier, st):
                if (tier, st) in st_wtile:
                    return
                V, K, w_ext, wpool, dr = tiers[tier]
                w = min(ST, V - st * ST)
                wtile = wpool.tile([P, K * ST], FP8,
                                   tag=f"w{tier}", name=f"w{tier}")
                for k in range(K):
                    nc.gpsimd.dma_start(
                        out=wtile[:, k * ST: k * ST + w],
                        in_=w_ext[k * P:(k + 1) * P, st * ST: st * ST + w])
                st_wtile[(tier, st)] = wtile

            def st_groups(tier, st):
                V = tiers[tier][0]
                w = min(ST, V - st * ST)
                return [(tier, st, g, min(GW, w - g * GW))
                        for g in range(_ceil_div(w, GW))]

            def emit_round(groups, rt, useV):
                base, cap = next_slot()
                off = 0
                for (tier, st, g, gw) in groups:
                    V, K, w_ext, wpool, dr = tiers[tier]
                    wtile = st_wtile[(tier, st)]
                    dst = mega[:, base + off: base + off + gw]
                    if dr:
                        wv = wtile[:].rearrange("p (k c) -> p k c", k=K)
                        lv = ht8v if tier == 0 else hp1Tv
                        for pr in range(K // 2):
                            nc.tensor.matmul(
                                out=dst,
                                lhsT=lv[:, 2 * pr: 2 * pr + 2,
                                        rt * P: rt * P + P],
                                rhs=wv[:, 2 * pr: 2 * pr + 2,
                                       g * GW: g * GW + gw],
                                start=(pr == 0), stop=(pr == K // 2 - 1),
                                perf_mode=DR)
                    else:
                        nc.tensor.matmul(
                            out=dst,
                            lhsT=hp2T_sb[:, rt * P: rt * P + P],
                            rhs=wtile[:, g * GW: g * GW + gw],
                            start=True, stop=True)
                    off += gw
                src = mega[:, base: base + off]
                if useV:
                    tier = groups[0][0]
                    e16 = e16pool.tile([P, 1536], I16, tag="e16")
                    nc.vector.tensor_scalar(
                        out=e16[:, :off], in0=src,
                        scalar1=EXP_C1, scalar2=EXP_C2[tier],
                        op0=ALU.mult, op1=ALU.add)
                    va = vacc[:, rt * VW: rt * VW + off]
                    if rt in G_RTS:
                        nc.gpsimd.tensor_tensor(
                            out=va, in0=va, in1=e16[:, :off].bitcast(BF16),
                            op=ALU.add)
                        eng_t["V"] += (off + 120) / 0.96 + 60.0
                    else:
                        nc.vector.tensor_tensor(
                            out=va, in0=va, in1=e16[:, :off].bitcast(BF16),
                            op=ALU.add)
                        eng_t["V"] += _cost_dve(off)
                else:
                    zcol = rt * ZC + zcols[rt]
                    zcols[rt] += 1
                    ex = expool.tile([P, 1536], BF16, tag="ex")
                    nc.scalar.activation(
                        ex[:, :off], src, ACTF.Exp,
                        accum_out=zbigA[:, zcol: zcol + 1])
                    eng_t["A"] += _cost_act(off)

            last_eng = ["A"]

            def plan_emit(tier_lists, rt):
                # per-tier queues; rounds draw via largest-remaining-fraction
                qs = [list(l) for l in tier_lists if l]
                tot = [len(q) for q in qs]
                while any(qs):
                    cap = CYCLE[slot_i[0] % 3][1]
                    nfit = cap // GW
                    # tier-pure candidate for a V round: tier with the most
                    # remaining groups
                    vi = max(range(len(qs)), key=lambda j: len(qs[j]))
                    vgroups = qs[vi][:nfit]
                    # A-round candidate: Bresenham across tiers
                    apick = []
                    idx = [0] * len(qs)
                    for _ in range(nfit):
                        best, bj = -1.0, -1
                        for j, q in enumerate(qs):
                            rema = len(q) - idx[j]
                            if rema > 0 and rema / tot[j] > best:
                                best, bj = rema / tot[j], j
                        if bj < 0:
                            break
                        apick.append((bj, idx[bj]))
                        idx[bj] += 1
                    agroups = [qs[j][k] for (j, k) in apick]
                    wV = sum(g[3] for g in vgroups)
                    wA = sum(g[3] for g in agroups)
                    # alternation bias against same-engine streaks
                    bias = 250.0 if last_eng[0] == "A" else -250.0
                    useV = bool(vgroups) and (
                        eng_t["V"] + _cost_dve(wV) - bias <
                        eng_t["A"] + _cost_act(wA))
                    if useV:
                        qs[vi] = qs[vi][nfit:]
                        emit_round(vgroups, rt, True)
                        last_eng[0] = "V"
                    else:
                        for (j, k) in sorted(apick, reverse=True):
                            qs[j].pop(k)
                        emit_round(agroups, rt, False)
                        last_eng[0] = "A"

            def emit_rt_final(rt):
                # row-tile Z reduction, emitted as soon as rt's stream ends
                nc.vector.tensor_reduce(
                    out=zredA[:, rt:rt + 1],
                    in_=zbigA[:, rt * ZC:(rt + 1) * ZC],
                    axis=mybir.AxisListType.X, op=ALU.add)
                eng_t["V"] += (ZC + 58) / 0.96 + 60
                cA = (VW + 224) / 1.2 + 181
                cV = (VW + 58) / 0.96
                if eng_t["A"] + cA < eng_t["V"] + cV:
                    ex = expool.tile([P, 1536], BF16, tag="ex")
                    nc.scalar.activation(
                        ex[:, :VW], vacc[:, rt * VW:(rt + 1) * VW],
                        ACTF.Identity, accum_out=zredV[:, rt:rt + 1])
                    eng_t["A"] += cA
                else:
                    nc.vector.tensor_reduce(
                        out=zredV[:, rt:rt + 1],
                        in_=vacc[:, rt * VW:(rt + 1) * VW],
                        axis=mybir.AxisListType.X, op=ALU.add)
                    eng_t["V"] += cV

            def emit_rows_proj(rt, t):
                # DR rows-orientation projection feeding the target dot
                pd = PD1 if t == 1 else PD2
                wv = wp18v if t == 1 else wp28v
                dstt = hp1r_sb if t == 1 else hp2r_sb
                base, cap = next_slot()
                for pr in range(4):
                    nc.tensor.matmul(
                        out=mega[:, base: base + pd],
                        lhsT=ht8v[:, 2 * pr: 2 * pr + 2,
                                  rt * P: rt * P + P],
                        rhs=wv[:, 2 * pr: 2 * pr + 2, 0:pd],
                        start=(pr == 0), stop=(pr == 3), perf_mode=DR)
                nc.vector.tensor_copy(
                    out=dstt[:, rt * pd:(rt + 1) * pd],
                    in_=mega[:, base: base + pd])
                eng_t["V"] += (pd + 120) / 0.96 + 60

            def emit_gather_dot(i):
                rt, t = divmod(i, 3)
                if t == 0:
                    hr_t = hrpool.tile([P, D], BF16, tag="hrt", name="hrt")
                    nc.sync.dma_start(out=hr_t[:],
                                      in_=hr_ext[rt * P:(rt + 1) * P, :])
                    feat_ap = hr_t[:]
                elif t == 1:
                    emit_rows_proj(rt, 1)
                    feat_ap = hp1r_sb[:, rt * PD1:(rt + 1) * PD1]
                else:
                    emit_rows_proj(rt, 2)
                    feat_ap = hp2r_sb[:, rt * PD2:(rt + 1) * PD2]
                g = gpool.tile([P, gdim[t]], BF16, tag=f"g{t}", name=f"g{t}")
                nc.gpsimd.indirect_dma_start(
                    out=g[:], out_offset=None,
                    in_=gather_src[t][:, :],
                    in_offset=IndirectOffsetOnAxis(
                        ap=idxi[t][:, rt:rt + 1], axis=0),
                    bounds_check=gmax[t], oob_is_err=False)
                prod = prodpool.tile([P, D], BF16, tag="prod")
                nc.vector.scalar_tensor_tensor(
                    out=prod[:, :gdim[t]],
                    in0=feat_ap, scalar=1.0, in1=g[:],
                    op0=ALU.mult, op1=ALU.mult,
                    accum_out=tl[t][:, rt:rt + 1])
                eng_t["V"] += (gdim[t] / 2 + 58) / 0.96 + 60

            def interleave(lists):
                # Bresenham-style proportional merge of per-tier group lists
                out = []
                idx = [0] * len(lists)
                tot = [len(l) for l in lists]
                n = sum(tot)
                for _ in range(n):
                    best, bi = -1.0, 0
                    for j, l in enumerate(lists):
                        if idx[j] < tot[j]:
                            frac = (tot[j] - idx[j]) / tot[j]
                            if frac > best:
                                best, bi = frac, j
                    out.append(lists[bi][idx[bi]])
                    idx[bi] += 1
                return out

            gi = 0
            for wi, (a_st, b_sts, c_sts) in enumerate(WINDOWS):
                for st in c_sts:
                    ensure_st(2, st)
                ensure_st(0, a_st)
                for st in b_sts:
                    ensure_st(1, st)
                As = st_groups(0, a_st)
                Bs = [g for st in b_sts for g in st_groups(1, st)]
                Cs = [g for st in c_sts for g in st_groups(2, st)]
                if wi == 0:
                    # runway: tier2 rounds only while w0/w1 land; vacc
                    # slices are zeroed here (V is otherwise idle early)
                    for rt in range(NRT):
                        # zero vacc on ScalarE (idle during the runway ramp)
                        nc.scalar.memzero(vacc[:, rt * VW:(rt + 1) * VW])
                        eng_t["A"] += (VW / 2 + 224) / 1.2 + 30
                        plan_emit([Cs[0:8]], rt)
                    # hp1T projection: needed by the first B rounds
                    for m in range(2):
                        base, cap = next_slot()
                        for g in range(2):
                            for pr in range(4):
                                nc.tensor.matmul(
                                    out=mega[:, base + g * GW:
                                             base + (g + 1) * GW],
                                    lhsT=wp18v[:, 2 * pr: 2 * pr + 2,
                                               m * P:(m + 1) * P],
                                    rhs=ht8v[:, 2 * pr: 2 * pr + 2,
                                             g * GW:(g + 1) * GW],
                                    start=(pr == 0), stop=(pr == 3),
                                    perf_mode=DR)
                        nc.vector.tensor_copy(
                            out=hp1T_sb[:, m * RPC:(m + 1) * RPC],
                            in_=mega[:, base: base + RPC])
                        eng_t["V"] += (RPC + 120) / 0.96 + 60
                    for rt in range(NRT):
                        plan_emit([Cs[8:], As, Bs], rt)
                    continue
                for rt in range(NRT):
                    plan_emit([As, Bs, Cs], rt)
                    if gi < 3 * NRT:
                        emit_gather_dot(gi)
                        gi += 1
                    if wi == 3:
                        emit_rt_final(rt)
            while gi < 3 * NRT:
                emit_gather_dot(gi)
                gi += 1

            # ---------------- final reduction ----------------
            # zred = zredA + zredV + d1 (d1 holds the ScalarE-reduced
            # second vacc half where that path was taken)
            nc.vector.tensor_tensor(out=zred[:], in0=zredA[:], in1=zredV[:],
                                    op=ALU.add)
            nc.scalar.activation(logz[:], zred[:], ACTF.Ln)
            # loss8 = logz - (tl0 + ge1*(tl1-tl0) + ge2*(tl2-tl1))
            nc.vector.tensor_tensor(out=d1[:], in0=tl[1][:], in1=tl[0][:],
                                    op=ALU.subtract)
            nc.vector.tensor_tensor(out=d2[:], in0=tl[2][:], in1=tl[1][:],
                                    op=ALU.subtract)
            nc.vector.tensor_tensor(out=d1[:], in0=d1[:], in1=ge1[:],
                                    op=ALU.mult)
            nc.vector.tensor_tensor(out=d2[:], in0=d2[:], in1=ge2[:],
                                    op=ALU.mult)
            nc.vector.tensor_tensor(out=loss8[:], in0=logz[:], in1=tl[0][:],
                                    op=ALU.subtract)
            nc.vector.tensor_tensor(out=loss8[:], in0=loss8[:], in1=d1[:],
                                    op=ALU.subtract)
            nc.vector.tensor_tensor(out=loss8[:], in0=loss8[:], in1=d2[:],
                                    op=ALU.subtract)
            nc.vector.tensor_reduce(out=lossv[:], in_=loss8[:],
                                    axis=mybir.AxisListType.X, op=ALU.add)
            base, cap = next_slot()
            nc.tensor.matmul(out=mega[0:1, base:base + 1], lhsT=lossv[:],
                             rhs=ones[:], start=True, stop=True)
            nc.scalar.mul(part[0:1, 0:1], mega[0:1, base:base + 1],
                          1.0 / float(B_T))
            nc.sync.dma_start(out=out_ext[:, :], in_=part[:])

    nc.compile()
    return nc


def _get_nc():
    global _NC_CACHE
    if _NC_CACHE is None:
        _NC_CACHE = _build_graph()
    return _NC_CACHE


def _make_in_maps(h, targets, W_head0, W_proj1, W_head1, W_proj2, W_head2):
    FP8NP = ml_dtypes.float8_e4m3
    BF16NP = ml_dtypes.bfloat16
    h = np.ascontiguousarray(np.asarray(h, dtype=np.float32)).reshape(B_T, D)
    t = np.asarray(targets).reshape(-1).astype(np.float32)
    w0 = np.asarray(W_head0, dtype=np.float32)
    w1 = np.asarray(W_head1, dtype=np.float32)
    w2 = np.asarray(W_head2, dtype=np.float32)
    wp1 = np.asarray(W_proj1, dtype=np.float32)
    wp2 = np.asarray(W_proj2, dtype=np.float32)
    w0_8 = np.ascontiguousarray(w0.astype(FP8NP))
    w1_8 = np.ascontiguousarray(w1.astype(FP8NP))
    w2_8 = np.ascontiguousarray(w2.astype(FP8NP))
    wp1_c = np.ascontiguousarray(
        wp1.astype(FP8NP).reshape(8, P, PD1).transpose(1, 0, 2).reshape(
            P, 8 * PD1))
    wp2_c = np.ascontiguousarray(
        wp2.astype(FP8NP).reshape(8, P, PD2).transpose(1, 0, 2).reshape(
            P, 8 * PD2))
    wt0 = np.ascontiguousarray(w0.T.astype(BF16NP))
    wt1 = np.ascontiguousarray(w1.T.astype(BF16NP))
    wt2 = np.ascontiguousarray(w2.T.astype(BF16NP))

    in_maps = []
    for c in range(N_CORES):
        hc = h[c * RPC:(c + 1) * RPC]
        tc_ = t[c * RPC:(c + 1) * RPC]
        ht8 = hc.T.astype(FP8NP).reshape(8, P, RPC).transpose(1, 0, 2)
        in_maps.append({
            "ht": np.ascontiguousarray(ht8.reshape(P, 8 * RPC)),
            "hr": np.ascontiguousarray(hc.astype(BF16NP)),
            "tf": np.ascontiguousarray(tc_.reshape(NRT, P).T),
            "wp1": wp1_c, "wp2": wp2_c,
            "w0": w0_8, "w1": w1_8, "w2": w2_8,
            "wt0": wt0, "wt1": wt1, "wt2": wt2,
        })
    return in_maps


def _finalize(results):
    total = sum(float(results[c]["out"][0, 0]) for c in range(N_CORES))
    return np.float32(total)


def kernel(h, targets, token_to_tier, token_to_idx,
           W_head0, W_proj1, W_head1, W_proj2, W_head2):
    in_maps = _make_in_maps(h, targets, W_head0, W_proj1, W_head1,
                            W_proj2, W_head2)
    nc = _get_nc()
    res = run_bass_kernel_spmd(nc, in_maps, core_ids=list(range(N_CORES)))
    return _finalize(res.results)
